# revision 1
# baseline (speedup 1.0000x reference)
"""Trainium2 Bass kernel for a cross-attention layer (CoAttention + RMSNorm output).

Reference computation (per batch b):
    q = hidden @ Wq.T + bq ; k = kv @ Wk.T + bk ; v = kv @ Wv.T + bv
    probs = softmax(q k^T / sqrt(64))
    ctx = probs @ v
    out = RMSNorm(ctx @ Wo.T + bo + hidden) * gamma

Sharding: 8 cores = 4 batches x 2 query-row halves. Each core produces
out[b, half*1024:(half+1)*1024, :] with no cross-core communication
(K/V projections are duplicated within a batch pair).

Per-core pipeline (all matmuls keep contraction dim on SBUF partitions,
enabled by host-side transposes of activations/weights):
  A) QT[o, s]  = WqT.T @ xqT          (fp32r, +bq via per-partition bias)
  B) KT[o, s]  = WkT.T @ xkvT -> DRAM scratch (streamed back per head pair)
  C) V[s, o]   = xkvT.T @ WvT -> SBUF resident as bf16 [kv, head, 64+1]
                 (65th column = 1.0: folds softmax row-sums into ctx matmul)
  D) per head: S^T[kv, sq] = KT_h.T @ QT_h (fp32r; two heads packed in the
     128-row PE array via base-partition 0/64), E = exp(S^T/8) in bf16 on ACT,
     ctx+^T[65, sq] += V+_h.T @ E (bf16); row 64 accumulates sum(exp).
     Normalize: R = broadcast(1/rowsum) via a K=1 PE matmul; ctxT = ctx+ * R.
  E) O[s, o] = ctxT.T @ WoT (bf16) + resid(+bo); RMSNorm * gamma; DMA out.
"""

import numpy as np
import ml_dtypes

import concourse.bass as bass
import concourse.mybir as mybir
from concourse import bass_utils, tile

P = 128
H = 1024
NH = 16
HD = 64
B = 4
SQ = 2048
SQL = 1024  # per-core query rows
SKV = 2048
KC = H // P  # 8 contraction chunks of 128
NKV = SKV // P  # 16 kv chunks
EPS = 1e-6

F32 = mybir.dt.float32
F32R = mybir.dt.float32r
BF16 = mybir.dt.bfloat16
AF = mybir.ActivationFunctionType
OP = mybir.AluOpType

N_CORES = 8


class SplitDrainTileContext(tile.TileContext):
    """TileContext whose tail drain splits sem waits across chained drains.

    The walrus build in this container rejects CTRL instructions that carry
    more than one sync wait; the stock tail drain aggregates the whole global
    clock onto a single Drain instruction.
    """

    MAXW = 1

    def _drain_and_barrier(self, tick_clock, wait_clock):
        drain_inst = self.nc.sync.drain()
        wait_clock.add_sem_waits(
            drain_inst.ins, tile.ScopedClock({None: tick_clock.global_clock})
        )
        si = drain_inst.ins.sync_info
        if si is not None and si.on_wait and len(si.on_wait) > self.MAXW:
            waits = list(si.on_wait)
            drain_inst.ins.sync_info = mybir.SyncInfo(
                on_wait=waits[: self.MAXW], on_update=list(si.on_update or [])
            )
            rest = waits[self.MAXW :]
            for i in range(0, len(rest), self.MAXW):
                d2 = self.nc.sync.drain()
                d2.ins.sync_info = mybir.SyncInfo(
                    on_wait=rest[i : i + self.MAXW], on_update=[]
                )
        self.nc.all_engine_barrier()
        assert self.sems is not None
        popped = self.nc._tile_sem_poison_stack.pop()
        assert popped is self._sem_poison
        self.nc.clear_and_free_semaphores(list(self.sems.allocated().values()))
        self.nc.all_engine_barrier()


def _split_sync_waits(nc, maxw=1):
    """Hoist excess per-instruction sem waits onto preceding same-engine NoOps.

    The walrus build in this container rejects instructions carrying more
    than one sync wait command (any opcode family)."""
    n = 0
    tail_bb = nc.cur_bb.bb
    for f in nc.m.functions:
        for bb in f.blocks:
            il = bb.instructions
            i = 0
            while i < len(il):
                inst = il[i]
                si = inst.sync_info
                if si is not None and si.on_wait and len(si.on_wait) > maxw:
                    waits = list(si.on_wait)
                    keep = waits[-maxw:]
                    extra = waits[:-maxw]
                    inst.sync_info = mybir.SyncInfo(
                        on_wait=keep, on_update=list(si.on_update or [])
                    )
                    for w in extra:
                        b = nc.engines[inst.engine].nop(nofuse=True)
                        carrier = b.ins
                        popped = tail_bb.instructions.pop()
                        assert popped is carrier, "nop landed in unexpected block"
                        carrier.sync_info = mybir.SyncInfo(on_wait=[w], on_update=[])
                        il.insert(i, carrier)
                        i += 1
                        n += 1
                i += 1
    return n


def build_core_kernel(split_waits=True):
    nc = bass.Bass("TRN2", target_bir_lowering=False, debug=False, num_devices=1)

    def inp(name, shape, dt=F32):
        return nc.dram_tensor(name, shape, dt, kind="ExternalInput").ap()

    xqT = inp("xqT", [H, SQL], BF16)
    xkvT = inp("xkvT", [H, SKV], BF16)
    wqT = inp("wqT", [H, H], BF16)
    wkT = inp("wkT", [H, H], BF16)
    wvT = inp("wvT", [H, H], BF16)
    woT = inp("woT", [H, H], BF16)
    bqc = inp("bqc", [P, KC])
    bkc = inp("bkc", [P, KC])
    bvr = inp("bvr", [P, H])
    resid = inp("resid", [SQL, H])
    gam = inp("gam", [P, H])
    onesd = inp("onesd", [1, HD], F32R)
    out = nc.dram_tensor("out", [SQL, H], F32, kind="ExternalOutput").ap()

    with SplitDrainTileContext(nc) as tc:
        with (
            nc.allow_low_precision(reason="bf16 staging of fp32 data"),
            tc.tile_pool(name="pers", bufs=1) as pers,
            tc.tile_pool(name="wt", bufs=2) as wpool,
            tc.tile_pool(name="xs", bufs=2) as xpool,
            tc.tile_pool(name="sm", bufs=8) as smpool,
            tc.tile_pool(name="r4", bufs=3) as rpool,
            tc.tile_pool(name="tiny", bufs=4) as tpool,
            tc.tile_pool(name="psc", bufs=2, space="PSUM") as spool,
            tc.tile_pool(name="pcx", bufs=2, space="PSUM") as cpool,
            tc.tile_pool(name="pmi", bufs=2, space="PSUM") as mpool,
        ):
            # --- persistent tiles -------------------------------------------------
            qt = pers.tile([P, KC, SQL], BF16, name="qt")          # Q^T  [o, s]
            ktall = pers.tile([P, KC, SKV], BF16, name="ktall")    # K^T  [o, s]
            v_sb = pers.tile([P, NKV, NH, HD + 1], BF16, name="v_sb")
            ctxT = pers.tile([P, KC, SQL], BF16, name="ctxT")      # ctx^T [c, s]
            accum_o = pers.tile([P, KC, H], BF16, name="accum_o")  # O partial sums
            bq_sb = pers.tile([P, KC], F32, name="bq_sb")
            bk_sb = pers.tile([P, KC], F32, name="bk_sb")
            bv_sb = pers.tile([P, H], F32, name="bv_sb")
            gam_sb = pers.tile([P, H], F32, name="gam_sb")
            ones1 = pers.tile([1, HD], F32R, name="ones1")
            eps_sb = pers.tile([P, 1], F32, name="eps_sb")
            nc.vector.memset(eps_sb, EPS)

            nc.sync.dma_start(bq_sb, bqc)
            nc.sync.dma_start(bk_sb, bkc)
            nc.sync.dma_start(bv_sb, bvr)
            nc.sync.dma_start(gam_sb, gam)
            nc.sync.dma_start(ones1, onesd)
            nc.vector.memset(v_sb[:, :, :, HD], 1.0)

            def load_w(wT, name):
                w = wpool.tile([P, KC, H], BF16, tag="wt", name=name)
                for ic in range(KC):
                    nc.sync.dma_start(w[:, ic, :], wT[ic * P : (ic + 1) * P, :])
                return w

            # --- phase A: Q^T = WqT.T @ xqT (+bq) ---------------------------------
            wq = load_w(wqT, "wq")
            for sc in range(SQL // 512):
                xq = xpool.tile([P, KC, 512], BF16, tag="xs", name="xq")
                for ic in range(KC):
                    nc.sync.dma_start(
                        xq[:, ic, :], xqT[ic * P : (ic + 1) * P, sc * 512 : (sc + 1) * 512]
                    )
                for oc in range(KC):
                    ps = cpool.tile([P, 512], F32, tag="pcx", name="ps_q")
                    for ic in range(KC):
                        nc.tensor.matmul(
                            ps,
                            wq[:, ic, oc * P : (oc + 1) * P],
                            xq[:, ic, :],
                            start=(ic == 0),
                            stop=(ic == KC - 1),
                        )
                    nc.vector.tensor_scalar_add(
                        qt[:, oc, sc * 512 : (sc + 1) * 512], ps, bq_sb[:, oc : oc + 1]
                    )

            # --- phase B: K^T = WkT.T @ xkvT (+bk), resident ----------------------
            wk = load_w(wkT, "wk")
            for sc in range(SKV // 512):
                xkv = xpool.tile([P, KC, 512], BF16, tag="xs", name="xkv")
                for ic in range(KC):
                    nc.sync.dma_start(
                        xkv[:, ic, :], xkvT[ic * P : (ic + 1) * P, sc * 512 : (sc + 1) * 512]
                    )
                for oc in range(KC):
                    ps = cpool.tile([P, 512], F32, tag="pcx", name="ps_k")
                    for ic in range(KC):
                        nc.tensor.matmul(
                            ps,
                            wk[:, ic, oc * P : (oc + 1) * P],
                            xkv[:, ic, :],
                            start=(ic == 0),
                            stop=(ic == KC - 1),
                        )
                    nc.vector.tensor_scalar_add(
                        ktall[:, oc, sc * 512 : (sc + 1) * 512], ps, bk_sb[:, oc : oc + 1]
                    )

            # --- phase C: V = xkvT.T @ WvT (+bv) -> bf16 SBUF ---------------------
            wv = load_w(wvT, "wv")

            def vproj_chunks(oc2, pool):
                """V-proj half; one closed PSUM group (8 MMs + copyback) per yield."""
                for sc in range(SKV // 512):
                    xkv2 = xpool.tile([P, KC, 512], BF16, tag="xs", name="xkv2")
                    for ic in range(KC):
                        nc.sync.dma_start(
                            xkv2[:, ic, :],
                            xkvT[ic * P : (ic + 1) * P, sc * 512 : (sc + 1) * 512],
                        )
                    for s2 in range(4):
                        kvc = sc * 4 + s2
                        ps = pool.tile([P, 512], F32, tag="pcx" if pool is cpool else "pmi", name="ps_v")
                        for ic in range(KC):
                            nc.tensor.matmul(
                                ps,
                                xkv2[:, ic, s2 * P : (s2 + 1) * P],
                                wv[:, ic, oc2 * 512 : (oc2 + 1) * 512],
                                start=(ic == 0),
                                stop=(ic == KC - 1),
                            )
                        nc.vector.tensor_tensor(
                            v_sb[:, kvc, oc2 * 8 : (oc2 + 1) * 8, 0:HD],
                            ps.rearrange("p (h d) -> p h d", d=HD),
                            bvr_view(bv_sb, oc2),
                            OP.add,
                        )
                        yield

            for _ in vproj_chunks(0, cpool):
                pass
            wo = load_w(woT, "wo")

            def ochunk_steps(cc):
                for s2 in range(SQL // P):
                    for oc2 in range(2):
                        po = mpool.tile([P, 512], F32, tag="pmi", name="ps_oc")
                        nc.tensor.matmul(
                            po,
                            ctxT[:, cc, s2 * P : (s2 + 1) * P],
                            wo[:, cc, oc2 * 512 : (oc2 + 1) * 512],
                            start=True,
                            stop=True,
                        )
                        dst = accum_o[:, s2, oc2 * 512 : (oc2 + 1) * 512]
                        if cc == 0:
                            nc.vector.tensor_copy(dst, po)
                        else:
                            nc.vector.tensor_tensor(dst, dst, po, OP.add)
                        yield

            def chain(*gens):
                for g in gens:
                    yield from g

            def pads(n):
                for _ in range(n):
                    yield

            def interleave(g, k):
                """yield one step of g, then k pad yields, until g is done."""
                for step in g:
                    yield
                    for _ in range(k):
                        yield

            # --- attention pair ---------------------------------------------------
            pending_norms = []

            def emit_pair(hp, filler):
                for sqc in range(2):
                    cps = [
                        cpool.tile([P, 512], F32, tag="pcx", name="ps_ctx")
                        for _ in range(2)
                    ]
                    prev_e = None

                    def emit_ctx(kvc, e):
                        for h in range(2):
                            nc.tensor.matmul(
                                cps[h][0 : HD + 1, :],
                                v_sb[:, kvc, 2 * hp + h, :],
                                e[:, h * 512 : (h + 1) * 512],
                                start=(kvc == 0),
                                stop=(kvc == NKV - 1),
                            )

                    for kvc in range(NKV):
                        if kvc == 1:
                            while pending_norms:
                                pending_norms.pop(0)()
                        sp = spool.tile([P, 1024], F32, tag="psc", name="ps_sc")
                        for h in range(2):
                            nc.tensor.matmul(
                                sp[:, h * 512 : (h + 1) * 512],
                                ktall[h * HD : (h + 1) * HD, hp, kvc * P : (kvc + 1) * P],
                                qt[
                                    h * HD : (h + 1) * HD,
                                    hp,
                                    sqc * 512 : (sqc + 1) * 512,
                                ],
                                start=True,
                                stop=True,
                            )
                        next(filler, None)
                        e = smpool.tile([P, 1024], BF16, tag="sm", name="e_t")
                        nc.scalar.activation(e, sp, AF.Exp, scale=0.125)
                        if prev_e is not None:
                            emit_ctx(kvc - 1, prev_e)
                        prev_e = e
                    emit_ctx(NKV - 1, prev_e)
                    # reciprocals now (DVE only); broadcast+multiply deferred so
                    # the R matmuls never block the next sq-half's scores on PE
                    recs = []
                    for h in range(2):
                        # 1/x as exp(-ln(x)) on ACT: keeps the slow DVE
                        # iterative divide off the boundary critical path
                        lnt = smpool.tile([1, 512], F32, tag="sm", name="lnt")
                        nc.scalar.activation(lnt, cps[h][HD : HD + 1, :], AF.Ln)
                        rec = smpool.tile([1, 512], F32R, tag="sm", name="rec")
                        nc.scalar.activation(rec, lnt, AF.Exp, scale=-1.0)
                        recs.append(rec)

                    def deferred_norm(hp=hp, sqc=sqc, cps=cps, recs=recs):
                        for h in range(2):
                            rp = mpool.tile([P, 512], F32, tag="pmi", name="ps_r")
                            nc.tensor.matmul(
                                rp[0:HD, :], ones1, recs[h], start=True, stop=True
                            )
                            r_sb = smpool.tile([HD, 512], F32, tag="sm", name="r_sb")
                            nc.vector.tensor_copy(r_sb, rp[0:HD, :])
                            dst = ctxT[
                                h * HD : (h + 1) * HD, hp, sqc * 512 : (sqc + 1) * 512
                            ]
                            if h == 0:
                                nc.vector.tensor_tensor(
                                    dst, cps[h][0:HD, :], r_sb, OP.mult
                                )
                            else:
                                stg = smpool.tile([HD, 512], BF16, tag="sm", name="stg")
                                nc.vector.tensor_tensor(
                                    stg, cps[h][0:HD, :], r_sb, OP.mult
                                )
                                nc.sync.dma_start(dst, stg)

                    pending_norms.append(deferred_norm)

            # --- schedule: pairs with background filler ---------------------------
            # positions (1-indexed, 32 per pair); och(cc) must start > 32*(cc+1)
            # och(cc) must start after pair cc's deferred norms (emitted at
            # kvc==1 of the following sq-half, i.e. step 32*(cc+1)+2)
            filler = chain(
                interleave(vproj_chunks(1, mpool), 1),  # 1-32: V1 over pair 0
                pads(6),
                ochunk_steps(0),                        # 39-54
                pads(16),
                ochunk_steps(1),                        # 69-84
                pads(16),
                ochunk_steps(2),                        # 101-116
                pads(16),
                ochunk_steps(3),                        # 133-148
                pads(16),
                ochunk_steps(4),                        # 165-180
                pads(16),
                ochunk_steps(5),                        # 197-212
                pads(16),
                ochunk_steps(6),                        # 229-244
            )
            for hp in range(NH // 2):
                emit_pair(hp, filler)
            while pending_norms:
                pending_norms.pop(0)()
            for _ in filler:
                pass
            for _ in ochunk_steps(NH // 2 - 1):
                pass

            # --- phase E: residual + RMSNorm epilogue -----------------------------
            for s2 in range(SQL // P):
                rs = rpool.tile([P, H], F32, tag="r4", name="rs")
                nc.sync.dma_start(rs, resid[s2 * P : (s2 + 1) * P, :])
                h_sb = rpool.tile([P, H], F32, tag="r4", name="h_sb")
                nc.vector.tensor_tensor(h_sb, accum_o[:, s2, :], rs, OP.add)
                sq = rpool.tile([P, H], F32, tag="r4", name="sq_scratch")
                ss = tpool.tile([P, 1], F32, tag="tiny", name="ss")
                nc.vector.tensor_tensor(sq, h_sb, h_sb, OP.mult)
                nc.vector.tensor_reduce(ss, sq, axis=mybir.AxisListType.X, op=OP.add)
                sr = tpool.tile([P, 1], F32, tag="tiny", name="sr")
                nc.scalar.activation(sr, ss, AF.Sqrt, scale=1.0 / H, bias=eps_sb)
                rr = tpool.tile([P, 1], F32, tag="tiny", name="rr")
                nc.vector.reciprocal(rr, sr)
                nc.vector.tensor_scalar_mul(h_sb, h_sb, rr)
                nc.vector.tensor_tensor(rs, h_sb, gam_sb, OP.mult)
                nc.sync.dma_start(out[s2 * P : (s2 + 1) * P, :], rs)

    if split_waits:
        _split_sync_waits(nc)
    return nc


def bvr_view(bv_sb, oc2):
    return bv_sb[:, oc2 * 512 : (oc2 + 1) * 512].rearrange("p (h d) -> p h d", d=HD)


_NC = None


def _get_nc():
    global _NC
    if _NC is None:
        _NC = build_core_kernel()
    return _NC


def make_in_maps(hidden_states, keyvalue_states, Wq, bq, Wk, bk, Wv, bv, Wo, bo, gamma):
    f = np.float32
    hidden_states = np.asarray(hidden_states, f)
    keyvalue_states = np.asarray(keyvalue_states, f)
    shared = {
        "wqT": np.ascontiguousarray(np.asarray(Wq, f).T).astype(ml_dtypes.bfloat16),
        "wkT": np.ascontiguousarray(np.asarray(Wk, f).T).astype(ml_dtypes.bfloat16),
        "wvT": np.ascontiguousarray(np.asarray(Wv, f).T).astype(ml_dtypes.bfloat16),
        "woT": np.ascontiguousarray(np.asarray(Wo, f).T).astype(ml_dtypes.bfloat16),
        "bqc": np.ascontiguousarray(np.asarray(bq, f).reshape(KC, P).T),
        "bkc": np.ascontiguousarray(np.asarray(bk, f).reshape(KC, P).T),
        "bvr": np.ascontiguousarray(np.tile(np.asarray(bv, f), (P, 1))),
        "gam": np.ascontiguousarray(np.tile(np.asarray(gamma, f), (P, 1))),
        "onesd": np.ones((1, HD), f),
    }
    bo = np.asarray(bo, f)
    in_maps = []
    for core in range(N_CORES):
        b, half = divmod(core, 2)
        hq = hidden_states[b, half * SQL : (half + 1) * SQL, :]
        m = dict(shared)
        m["xqT"] = np.ascontiguousarray(hq.T).astype(ml_dtypes.bfloat16)
        m["xkvT"] = np.ascontiguousarray(keyvalue_states[b].T).astype(ml_dtypes.bfloat16)
        m["resid"] = np.ascontiguousarray(hq + bo)
        in_maps.append(m)
    return in_maps


def _run(in_maps, trace=False, **kwargs):
    nc = _get_nc()
    return bass_utils.run_bass_kernel_spmd(
        nc, in_maps, core_ids=list(range(N_CORES)), trace=trace, **kwargs
    )


def _assemble(res):
    out = np.empty((B, SQ, H), np.float32)
    for core in range(N_CORES):
        b, half = divmod(core, 2)
        out[b, half * SQL : (half + 1) * SQL, :] = res.results[core]["out"]
    return out


def kernel(hidden_states, keyvalue_states, Wq, bq, Wk, bk, Wv, bv, Wo, bo, gamma):
    in_maps = make_in_maps(
        hidden_states, keyvalue_states, Wq, bq, Wk, bk, Wv, bv, Wo, bo, gamma
    )
    return _assemble(_run(in_maps))



# revision 6
# speedup vs baseline: 1.0687x; 1.0687x over previous
"""Trainium2 Bass kernel for a cross-attention layer (CoAttention + RMSNorm output).

Reference computation (per batch b):
    q = hidden @ Wq.T + bq ; k = kv @ Wk.T + bk ; v = kv @ Wv.T + bv
    probs = softmax(q k^T / sqrt(64))
    ctx = probs @ v
    out = RMSNorm(ctx @ Wo.T + bo + hidden) * gamma

Sharding: 8 cores = 4 batches x 2 query-row halves; no cross-core comms.

Numerics: everything upstream of the residual add runs in fp8(e4m3) with a
global x64 weight scale (W' = 64W). Scale bookkeeping:
  q' = x@Wq' = 64q (fp8), k' = 64k (fp8)   -> q'.k' = 4096 q.k
  exp scale 0.125/4096 recovers exp(q.k/8); e stored fp8 true-scale
  v' = 64v (fp8, ones column = 64)         -> ctx_psum = 64*ctx, row64 = 64*rowsum
  ctxT = ctx_psum * (64/row64) = 64*ctx_norm (fp8)
  O_psum = ctxT @ Wo' = 4096*O; resid is pre-scaled 4096(hidden+bo) on host
  RMSNorm(4096 h) == RMSNorm(h) with eps' = 4096^2 eps  (exact)

Engine plan (per core, target ~300us):
  PE  ~200us: fp8 DoubleRow (K=256/pass) projections Q/K/V/O + ctx; scores as
       two PE-row-tiled concurrent K=64 matmuls per kvc (216ns/pair warm).
       All projections stream as fillers inside the attention loop so the PE
       never idles (keeps the HAM clock-gate warm).
  ACT ~285us (pacer): one exp [128,1024] fp32 PSUM -> fp8 per kvc step.
  DVE: bias-adds, PSUM copybacks, softmax-denominator reciprocals, norms.
  GpSimd: gamma multiply in the RMSNorm epilogue.
"""

import numpy as np
import ml_dtypes

import concourse.bass as bass
import concourse.mybir as mybir
from concourse import bass_utils, tile

P = 128
H = 1024
NH = 16
HD = 64
B = 4
SQ = 2048
SQL = 1024  # per-core query rows
SKV = 2048
KC = H // P  # 8 contraction chunks of 128
NKV = SKV // P  # 16 kv chunks
NKT = NKV // 2  # kv chunk pairs (DoubleRow)
EPS = 1e-6

WS = 64.0  # weight scale
RS = WS * WS  # resid scale (4096)
EXPS = 0.125 / RS
EPS_HAT = EPS * RS * RS

F32 = mybir.dt.float32
BF16 = mybir.dt.bfloat16
FP8 = mybir.dt.float8e4
AF = mybir.ActivationFunctionType
OP = mybir.AluOpType
DR = mybir.MatmulPerfMode.DoubleRow

N_CORES = 8


class SplitDrainTileContext(tile.TileContext):
    """TileContext whose tail drain splits sem waits across chained drains.

    The walrus build in this container rejects CTRL instructions that carry
    more than one sync wait; the stock tail drain aggregates the whole global
    clock onto a single Drain instruction.
    """

    MAXW = 1

    def _drain_and_barrier(self, tick_clock, wait_clock):
        drain_inst = self.nc.sync.drain()
        wait_clock.add_sem_waits(
            drain_inst.ins, tile.ScopedClock({None: tick_clock.global_clock})
        )
        si = drain_inst.ins.sync_info
        if si is not None and si.on_wait and len(si.on_wait) > self.MAXW:
            waits = list(si.on_wait)
            drain_inst.ins.sync_info = mybir.SyncInfo(
                on_wait=waits[: self.MAXW], on_update=list(si.on_update or [])
            )
            rest = waits[self.MAXW :]
            for i in range(0, len(rest), self.MAXW):
                d2 = self.nc.sync.drain()
                d2.ins.sync_info = mybir.SyncInfo(
                    on_wait=rest[i : i + self.MAXW], on_update=[]
                )
        self.nc.all_engine_barrier()
        assert self.sems is not None
        popped = self.nc._tile_sem_poison_stack.pop()
        assert popped is self._sem_poison
        self.nc.clear_and_free_semaphores(list(self.sems.allocated().values()))
        self.nc.all_engine_barrier()


def _split_sync_waits(nc, maxw=1):
    """Hoist excess per-instruction sem waits onto preceding same-engine NoOps.

    The walrus build in this container rejects instructions carrying more
    than one sync wait command (any opcode family)."""
    n = 0
    tail_bb = nc.cur_bb.bb
    for f in nc.m.functions:
        for bb in f.blocks:
            il = bb.instructions
            i = 0
            while i < len(il):
                inst = il[i]
                si = inst.sync_info
                if si is not None and si.on_wait and len(si.on_wait) > maxw:
                    waits = list(si.on_wait)
                    keep = waits[-maxw:]
                    extra = waits[:-maxw]
                    inst.sync_info = mybir.SyncInfo(
                        on_wait=keep, on_update=list(si.on_update or [])
                    )
                    for w in extra:
                        b = nc.engines[inst.engine].nop(nofuse=True)
                        carrier = b.ins
                        popped = tail_bb.instructions.pop()
                        assert popped is carrier, "nop landed in unexpected block"
                        carrier.sync_info = mybir.SyncInfo(on_wait=[w], on_update=[])
                        il.insert(i, carrier)
                        i += 1
                        n += 1
                i += 1
    return n


def build_core_kernel(split_waits=True):
    nc = bass.Bass("TRN2", target_bir_lowering=False, debug=False, num_devices=1)

    def inp(name, shape, dt=F32):
        return nc.dram_tensor(name, shape, dt, kind="ExternalInput").ap()

    xqT = inp("xqT", [H, SQL], FP8)
    xkvT = inp("xkvT", [H, SKV], FP8)
    wqT = inp("wqT", [H, H], FP8)
    wkT = inp("wkT", [H, H], FP8)
    wvT = inp("wvT", [H, H], FP8)
    woT = inp("woT", [H, H], FP8)
    bqc = inp("bqc", [P, KC])
    bkc = inp("bkc", [P, KC])
    bvr = inp("bvr", [P, H])
    resid = inp("resid", [SQL, H])
    gam = inp("gam", [P, H])
    out = nc.dram_tensor("out", [SQL, H], F32, kind="ExternalOutput").ap()

    with SplitDrainTileContext(nc) as tc:
        with (
            nc.allow_low_precision(reason="fp8 attention at x64 scale"),
            tc.tile_pool(name="pers", bufs=1) as pers,
            tc.tile_pool(name="e2", bufs=3) as epool,
            tc.tile_pool(name="sm", bufs=6) as smpool,
            tc.tile_pool(name="rsd", bufs=3) as rspool,
            tc.tile_pool(name="hsb", bufs=2) as hpool,
            tc.tile_pool(name="sq", bufs=2) as sqpool,
            tc.tile_pool(name="tiny", bufs=6) as tpool,
            tc.tile_pool(name="psc", bufs=2, space="PSUM") as spool,
            tc.tile_pool(name="pcx", bufs=2, space="PSUM") as cpool,
            tc.tile_pool(name="pmi", bufs=2, space="PSUM") as mpool,
        ):
            # --- persistent tiles ------------------------------------------
            qt = pers.tile([P, KC, SQL], FP8, name="qt")          # 64*Q^T [o,s]
            ktall = pers.tile([P, KC, SKV], FP8, name="ktall")    # 64*K^T [o,s]
            v_sb = pers.tile([P, NKV, NH, HD + 1], FP8, name="v_sb")
            ctxT = pers.tile([P, KC, SQL], FP8, name="ctxT")      # 64*ctx^T
            accum_o = pers.tile([P, KC, H], BF16, name="accum_o")  # 4096*O
            wq_sb = pers.tile([P, KC, H], FP8, name="wq_sb")
            wk_sb = pers.tile([P, KC, H], FP8, name="wk_sb")
            wv_sb = pers.tile([P, KC, H], FP8, name="wv_sb")
            wo_sb = pers.tile([P, KC, H], FP8, name="wo_sb")
            xq_sb = pers.tile([P, KC, SQL], FP8, name="xq_sb")
            xkv_sb = pers.tile([P, KC, SKV], FP8, name="xkv_sb")
            bq_sb = pers.tile([P, KC], F32, name="bq_sb")
            bk_sb = pers.tile([P, KC], F32, name="bk_sb")
            bv_sb = pers.tile([P, H], F32, name="bv_sb")
            gam_sb = pers.tile([P, H], F32, name="gam_sb")
            ones64 = pers.tile([1, HD], BF16, name="ones64")
            eps_sb = pers.tile([P, 1], F32, name="eps_sb")
            nc.vector.memset(eps_sb, EPS_HAT)
            nc.vector.memset(ones64, WS)
            nc.vector.memset(v_sb[:, :, :, HD], WS)

            # --- DMAs (order matters for startup latency) ------------------
            nc.sync.dma_start(bq_sb, bqc)
            nc.sync.dma_start(bk_sb, bkc)
            nc.sync.dma_start(bv_sb, bvr)
            nc.sync.dma_start(gam_sb, gam)
            for ic in range(KC):
                nc.sync.dma_start(wq_sb[:, ic, :], wqT[ic * P : (ic + 1) * P, :])
            for ic in range(KC):
                nc.sync.dma_start(
                    xq_sb[:, ic, 0:512], xqT[ic * P : (ic + 1) * P, 0:512]
                )
            for ic in range(KC):
                nc.sync.dma_start(wk_sb[:, ic, :], wkT[ic * P : (ic + 1) * P, :])
            for ic in range(KC):
                nc.sync.dma_start(xkv_sb[:, ic, :], xkvT[ic * P : (ic + 1) * P, :])
            for ic in range(KC):
                nc.sync.dma_start(wv_sb[:, ic, :], wvT[ic * P : (ic + 1) * P, :])
            for ic in range(KC):
                nc.sync.dma_start(
                    xq_sb[:, ic, 512:1024], xqT[ic * P : (ic + 1) * P, 512:1024]
                )
            for ic in range(KC):
                nc.sync.dma_start(wo_sb[:, ic, :], woT[ic * P : (ic + 1) * P, :])

            # --- projection quanta (generators yielding per-MM) ------------
            def proj_q(oc, sqc):
                ps = mpool.tile([P, 512], F32, tag="pmi", name="ps_q")
                for t in range(KC // 2):
                    nc.tensor.matmul(
                        ps,
                        wq_sb[:, 2 * t : 2 * t + 2, oc * P : (oc + 1) * P],
                        xq_sb[:, 2 * t : 2 * t + 2, sqc * 512 : (sqc + 1) * 512],
                        start=(t == 0),
                        stop=(t == KC // 2 - 1),
                        perf_mode=DR,
                    )
                    yield
                nc.vector.tensor_scalar_add(
                    qt[:, oc, sqc * 512 : (sqc + 1) * 512], ps, bq_sb[:, oc : oc + 1]
                )
                yield

            def proj_k(oc, sc):
                ps = mpool.tile([P, 512], F32, tag="pmi", name="ps_k")
                for t in range(KC // 2):
                    nc.tensor.matmul(
                        ps,
                        wk_sb[:, 2 * t : 2 * t + 2, oc * P : (oc + 1) * P],
                        xkv_sb[:, 2 * t : 2 * t + 2, sc * 512 : (sc + 1) * 512],
                        start=(t == 0),
                        stop=(t == KC // 2 - 1),
                        perf_mode=DR,
                    )
                    yield
                nc.vector.tensor_scalar_add(
                    ktall[:, oc, sc * 512 : (sc + 1) * 512], ps, bk_sb[:, oc : oc + 1]
                )
                yield

            def proj_v(kvc, half):
                ps = mpool.tile([P, 512], F32, tag="pmi", name="ps_v")
                for t in range(KC // 2):
                    nc.tensor.matmul(
                        ps,
                        xkv_sb[:, 2 * t : 2 * t + 2, kvc * P : (kvc + 1) * P],
                        wv_sb[:, 2 * t : 2 * t + 2, half * 512 : (half + 1) * 512],
                        start=(t == 0),
                        stop=(t == KC // 2 - 1),
                        perf_mode=DR,
                    )
                    yield
                nc.vector.tensor_tensor(
                    v_sb[:, kvc, half * 8 : (half + 1) * 8, 0:HD],
                    ps.rearrange("p (h d) -> p h d", d=HD),
                    bv_sb[:, half * 512 : (half + 1) * 512].rearrange(
                        "p (h d) -> p h d", d=HD
                    ),
                    OP.add,
                )
                yield

            def proj_o_dr(t):
                # one DoubleRow pass over cc pair (2t, 2t+1), all 16 out tiles
                for s2 in range(KC):
                    for oc2 in range(2):
                        po = mpool.tile([P, 512], F32, tag="pmi", name="ps_o")
                        nc.tensor.matmul(
                            po,
                            ctxT[:, 2 * t : 2 * t + 2, s2 * P : (s2 + 1) * P],
                            wo_sb[:, 2 * t : 2 * t + 2, oc2 * 512 : (oc2 + 1) * 512],
                            start=True,
                            stop=True,
                            perf_mode=DR,
                        )
                        dst = accum_o[:, s2, oc2 * 512 : (oc2 + 1) * 512]
                        if t == 0:
                            nc.vector.tensor_copy(dst, po)
                        else:
                            nc.vector.tensor_tensor(dst, dst, po, OP.add)
                        yield

            def proj_o_single(cc):
                for s2 in range(KC):
                    for oc2 in range(2):
                        po = mpool.tile([P, 512], F32, tag="pmi", name="ps_o1")
                        nc.tensor.matmul(
                            po,
                            ctxT[:, cc, s2 * P : (s2 + 1) * P],
                            wo_sb[:, cc, oc2 * 512 : (oc2 + 1) * 512],
                            start=True,
                            stop=True,
                        )
                        dst = accum_o[:, s2, oc2 * 512 : (oc2 + 1) * 512]
                        nc.vector.tensor_tensor(dst, dst, po, OP.add)
                        yield

            def resid_dma(c):
                rs = rspool.tile([P, H], F32, tag="rsd", name="rs")
                nc.sync.dma_start(rs, resid[c * P : (c + 1) * P, :])
                resid_tiles[c] = rs
                yield

            resid_tiles = {}

            def chain(*gens):
                for g in gens:
                    yield from g

            def pads(n):
                for _ in range(n):
                    yield

            # --- filler schedule (iter = (hp, sqc), 16 kvc steps each) -----
            # budgets: iter0: 6/step, iter1: 4/step, else 3/step
            def iter_filler(it):
                if it == 0:
                    return chain(
                        proj_k(0, 1), proj_v(2, 0), proj_v(3, 0),
                        proj_k(0, 2), proj_v(4, 0), proj_v(5, 0),
                        proj_k(0, 3), proj_v(6, 0), proj_v(7, 0),
                        proj_v(8, 0), proj_v(9, 0), proj_q(0, 1),
                        proj_v(10, 0), proj_v(11, 0), proj_v(12, 0),
                        proj_v(13, 0), proj_v(14, 0), proj_v(15, 0),
                    )
                if it == 1:
                    return chain(
                        proj_k(1, 0), proj_k(1, 1), proj_k(1, 2), proj_k(1, 3),
                        proj_q(1, 0), proj_q(1, 1),
                        proj_v(0, 1), proj_v(1, 1), proj_v(2, 1), proj_v(3, 1),
                        proj_v(4, 1), proj_v(5, 1),
                    )
                if it == 2:
                    return chain(
                        proj_k(2, 0), proj_k(2, 1), proj_k(2, 2), proj_k(2, 3),
                        proj_q(2, 0), proj_q(2, 1),
                        proj_v(6, 1), proj_v(7, 1), proj_v(8, 1),
                    )
                if it == 3:
                    return chain(
                        proj_v(9, 1), proj_v(10, 1), proj_v(11, 1), proj_v(12, 1),
                        proj_v(13, 1), proj_v(14, 1), proj_v(15, 1),
                    )
                if it == 4:
                    return chain(
                        proj_k(3, 0), proj_k(3, 1), proj_k(3, 2), proj_k(3, 3),
                        proj_q(3, 0), proj_q(3, 1),
                    )
                if it == 5:
                    return proj_o_dr(0)
                if it == 6:
                    return chain(
                        proj_k(4, 0), proj_k(4, 1), proj_k(4, 2), proj_k(4, 3),
                        proj_q(4, 0), proj_q(4, 1),
                    )
                if it == 8:
                    return chain(
                        proj_k(5, 0), proj_k(5, 1), proj_k(5, 2), proj_k(5, 3),
                        proj_q(5, 0), proj_q(5, 1),
                    )
                if it == 9:
                    return proj_o_dr(1)
                if it == 10:
                    return chain(
                        proj_k(6, 0), proj_k(6, 1), proj_k(6, 2), proj_k(6, 3),
                        proj_q(6, 0), proj_q(6, 1),
                    )
                if it == 12:
                    return chain(
                        proj_k(7, 0), proj_k(7, 1), proj_k(7, 2), proj_k(7, 3),
                        proj_q(7, 0), proj_q(7, 1),
                    )
                if it == 13:
                    return proj_o_dr(2)
                if it == 14:
                    return chain(
                        pads(6),  # pair-6 sqc1 norms drain at step 1
                        proj_o_single(6),
                        resid_dma(0),
                    )
                if it == 15:
                    return chain(resid_dma(1), resid_dma(2))
                return pads(0)

            # --- prologue: just enough for pair 0 to start -----------------
            for _ in proj_q(0, 0):
                pass
            for _ in proj_k(0, 0):
                pass
            for _ in proj_v(0, 0):
                pass
            for _ in proj_v(1, 0):
                pass

            # --- attention main loop ---------------------------------------
            pending_norms = []

            def emit_pair(hp, sqc, it, filler, budget):
                cps = [
                    cpool.tile([P, 512], F32, tag="pcx", name="ps_ctx")
                    for _ in range(2)
                ]

                def emit_ctx(kt, e2):
                    for h in range(2):
                        nc.tensor.matmul(
                            cps[h][0 : HD + 1, :],
                            v_sb[:, 2 * kt : 2 * kt + 2, 2 * hp + h, :],
                            e2[:, :, h * 512 : (h + 1) * 512],
                            start=(kt == 0),
                            stop=(kt == NKT - 1),
                            perf_mode=DR,
                        )

                prev = None
                for kt in range(NKT):
                    e2 = epool.tile([P, 2, 1024], FP8, tag="e2", name="e2")
                    for j in range(2):
                        kvc = 2 * kt + j
                        if kvc == 1:
                            while pending_norms:
                                pending_norms.pop(0)()
                        sp = spool.tile([P, 1024], F32, tag="psc", name="ps_sc")
                        for h in range(2):
                            nc.tensor.matmul(
                                sp[:, h * 512 : (h + 1) * 512],
                                ktall[
                                    h * HD : (h + 1) * HD, hp, kvc * P : (kvc + 1) * P
                                ],
                                qt[
                                    h * HD : (h + 1) * HD,
                                    hp,
                                    sqc * 512 : (sqc + 1) * 512,
                                ],
                                start=True,
                                stop=True,
                            )
                        for _ in range(budget):
                            next(filler, None)
                        nc.scalar.activation(e2[:, j, :], sp, AF.Exp, scale=EXPS)
                    if prev is not None:
                        emit_ctx(kt - 1, prev)
                    prev = e2
                emit_ctx(NKT - 1, prev)

                # softmax denominators: DVE reciprocal (off the ACT stream)
                recs = []
                for h in range(2):
                    rec_f = smpool.tile([1, 512], F32, tag="sm", name="rec_f")
                    nc.vector.reciprocal(rec_f, cps[h][HD : HD + 1, :])
                    rec = smpool.tile([1, 512], BF16, tag="sm", name="rec")
                    nc.vector.tensor_copy(rec, rec_f)
                    recs.append(rec)

                def deferred_norm(hp=hp, sqc=sqc, cps=cps, recs=recs):
                    for h in range(2):
                        rp = mpool.tile([P, 512], F32, tag="pmi", name="ps_r")
                        nc.tensor.matmul(
                            rp[0:HD, :], ones64, recs[h], start=True, stop=True
                        )
                        r_sb = smpool.tile([HD, 512], F32, tag="sm", name="r_sb")
                        nc.vector.tensor_copy(r_sb, rp[0:HD, :])
                        dst = ctxT[
                            h * HD : (h + 1) * HD, hp, sqc * 512 : (sqc + 1) * 512
                        ]
                        if h == 0:
                            nc.vector.tensor_tensor(
                                dst, cps[h][0:HD, :], r_sb, OP.mult
                            )
                        else:
                            stg = smpool.tile([HD, 512], FP8, tag="sm", name="stg")
                            nc.vector.tensor_tensor(
                                stg, cps[h][0:HD, :], r_sb, OP.mult
                            )
                            nc.sync.dma_start(dst, stg)

                pending_norms.append(deferred_norm)

            for it in range(16):
                hp, sqc = divmod(it, 2)
                budget = 6 if it == 0 else (4 if it == 1 else 3)
                filler = iter_filler(it)
                emit_pair(hp, sqc, it, filler, budget)
                for _ in filler:  # drain any leftovers
                    pass

            while pending_norms:
                pending_norms.pop(0)()
            for _ in proj_o_single(7):
                pass

            # --- RMSNorm epilogue (rs prefetch pipelined 3 deep) -----------
            for c in range(KC):
                if c + 3 < KC:
                    for _ in resid_dma(c + 3):
                        pass
                rs = resid_tiles[c]
                h_sb = hpool.tile([P, H], F32, tag="hsb", name="h_sb")
                nc.vector.tensor_tensor(h_sb, accum_o[:, c, :], rs, OP.add)
                sq = sqpool.tile([P, H], BF16, tag="sq", name="sq_scr")
                ss = tpool.tile([P, 1], F32, tag="tiny", name="ss")
                nc.scalar.activation(sq, h_sb, AF.Square, accum_out=ss)
                sr = tpool.tile([P, 1], F32, tag="tiny", name="sr")
                nc.scalar.activation(sr, ss, AF.Sqrt, scale=1.0 / H, bias=eps_sb)
                rr = tpool.tile([P, 1], F32, tag="tiny", name="rr")
                nc.vector.reciprocal(rr, sr)
                nc.vector.tensor_scalar_mul(h_sb, h_sb, rr)
                nc.gpsimd.tensor_tensor(rs, h_sb, gam_sb, OP.mult)
                nc.sync.dma_start(out[c * P : (c + 1) * P, :], rs)

    if split_waits:
        _split_sync_waits(nc)
    return nc


_NC = None


def _get_nc():
    global _NC
    if _NC is None:
        _NC = build_core_kernel()
    return _NC


def _fp8(a):
    return np.clip(a, -240.0, 240.0).astype(ml_dtypes.float8_e4m3)


def make_in_maps(hidden_states, keyvalue_states, Wq, bq, Wk, bk, Wv, bv, Wo, bo, gamma):
    f = np.float32
    hidden_states = np.asarray(hidden_states, f)
    keyvalue_states = np.asarray(keyvalue_states, f)
    shared = {
        "wqT": _fp8(np.asarray(Wq, f).T * WS),
        "wkT": _fp8(np.asarray(Wk, f).T * WS),
        "wvT": _fp8(np.asarray(Wv, f).T * WS),
        "woT": _fp8(np.asarray(Wo, f).T * WS),
        "bqc": np.ascontiguousarray(np.asarray(bq, f).reshape(KC, P).T * WS),
        "bkc": np.ascontiguousarray(np.asarray(bk, f).reshape(KC, P).T * WS),
        "bvr": np.ascontiguousarray(np.tile(np.asarray(bv, f) * WS, (P, 1))),
        "gam": np.ascontiguousarray(np.tile(np.asarray(gamma, f), (P, 1))),
    }
    bo = np.asarray(bo, f)
    in_maps = []
    for core in range(N_CORES):
        b, half = divmod(core, 2)
        hq = hidden_states[b, half * SQL : (half + 1) * SQL, :]
        m = dict(shared)
        m["xqT"] = _fp8(hq.T)
        m["xkvT"] = _fp8(keyvalue_states[b].T)
        m["resid"] = np.ascontiguousarray((hq + bo) * RS)
        in_maps.append(m)
    return in_maps


def _run(in_maps, trace=False, **kwargs):
    nc = _get_nc()
    return bass_utils.run_bass_kernel_spmd(
        nc, in_maps, core_ids=list(range(N_CORES)), trace=trace, **kwargs
    )


def _assemble(res):
    out = np.empty((B, SQ, H), np.float32)
    for core in range(N_CORES):
        b, half = divmod(core, 2)
        out[b, half * SQL : (half + 1) * SQL, :] = res.results[core]["out"]
    return out


def kernel(hidden_states, keyvalue_states, Wq, bq, Wk, bk, Wv, bv, Wo, bo, gamma):
    in_maps = make_in_maps(
        hidden_states, keyvalue_states, Wq, bq, Wk, bk, Wv, bv, Wo, bo, gamma
    )
    return _assemble(_run(in_maps))


# revision 8
# speedup vs baseline: 1.1652x; 1.0903x over previous
"""Trainium2 Bass kernel for a cross-attention layer (CoAttention + RMSNorm output).

Reference computation (per batch b):
    q = hidden @ Wq.T + bq ; k = kv @ Wk.T + bk ; v = kv @ Wv.T + bv
    probs = softmax(q k^T / sqrt(64))
    ctx = probs @ v
    out = RMSNorm(ctx @ Wo.T + bo + hidden) * gamma

Sharding: 8 cores = 4 batches x 2 query-row halves; no cross-core comms.

Numerics: everything upstream of the residual add runs in fp8(e4m3) with a
global x64 weight scale (W' = 64W). Scale bookkeeping:
  q' = x@Wq' = 64q (fp8), k' = 64k (fp8)   -> q'.k' = 4096 q.k
  exp scale 0.125/4096 recovers exp(q.k/8); e stored fp8 true-scale
  v' = 64v (fp8, ones column = 64)         -> ctx_psum = 64*ctx, row64 = 64*rowsum
  ctxT = ctx_psum * (64/row64) = 64*ctx_norm (fp8)
  O_psum = ctxT @ Wo' = 4096*O; resid is pre-scaled 4096(hidden+bo) on host
  RMSNorm(4096 h) == RMSNorm(h) with eps' = 4096^2 eps  (exact)

Engine plan (per core, target ~300us):
  PE  ~200us: fp8 DoubleRow (K=256/pass) projections Q/K/V/O + ctx; scores as
       two PE-row-tiled concurrent K=64 matmuls per kvc (216ns/pair warm).
       All projections stream as fillers inside the attention loop so the PE
       never idles (keeps the HAM clock-gate warm).
  ACT ~285us (pacer): one exp [128,1024] fp32 PSUM -> fp8 per kvc step.
  DVE: bias-adds, PSUM copybacks, softmax-denominator reciprocals, norms.
  GpSimd: gamma multiply in the RMSNorm epilogue.
"""

import numpy as np
import ml_dtypes

import concourse.bass as bass
import concourse.mybir as mybir
from concourse import bass_utils, tile

P = 128
H = 1024
NH = 16
HD = 64
B = 4
SQ = 2048
SQL = 1024  # per-core query rows
SKV = 2048
KC = H // P  # 8 contraction chunks of 128
NKV = SKV // P  # 16 kv chunks
NKT = NKV // 2  # kv chunk pairs (DoubleRow)
EPS = 1e-6

WS = 64.0  # weight scale
RS = WS * WS  # resid scale (4096)
EXPS = 0.125 / RS
EPS_HAT = EPS * RS * RS

F32 = mybir.dt.float32
BF16 = mybir.dt.bfloat16
FP8 = mybir.dt.float8e4
AF = mybir.ActivationFunctionType
OP = mybir.AluOpType
DR = mybir.MatmulPerfMode.DoubleRow

N_CORES = 8


class SplitDrainTileContext(tile.TileContext):
    """TileContext whose tail drain splits sem waits across chained drains.

    The walrus build in this container rejects CTRL instructions that carry
    more than one sync wait; the stock tail drain aggregates the whole global
    clock onto a single Drain instruction.
    """

    MAXW = 1

    def _drain_and_barrier(self, tick_clock, wait_clock):
        drain_inst = self.nc.sync.drain()
        wait_clock.add_sem_waits(
            drain_inst.ins, tile.ScopedClock({None: tick_clock.global_clock})
        )
        si = drain_inst.ins.sync_info
        if si is not None and si.on_wait and len(si.on_wait) > self.MAXW:
            waits = list(si.on_wait)
            drain_inst.ins.sync_info = mybir.SyncInfo(
                on_wait=waits[: self.MAXW], on_update=list(si.on_update or [])
            )
            rest = waits[self.MAXW :]
            for i in range(0, len(rest), self.MAXW):
                d2 = self.nc.sync.drain()
                d2.ins.sync_info = mybir.SyncInfo(
                    on_wait=rest[i : i + self.MAXW], on_update=[]
                )
        self.nc.all_engine_barrier()
        assert self.sems is not None
        popped = self.nc._tile_sem_poison_stack.pop()
        assert popped is self._sem_poison
        self.nc.clear_and_free_semaphores(list(self.sems.allocated().values()))
        self.nc.all_engine_barrier()


def _split_sync_waits(nc, maxw=1):
    """Hoist excess per-instruction sem waits onto preceding same-engine NoOps.

    The walrus build in this container rejects instructions carrying more
    than one sync wait command (any opcode family)."""
    n = 0
    tail_bb = nc.cur_bb.bb
    for f in nc.m.functions:
        for bb in f.blocks:
            il = bb.instructions
            i = 0
            while i < len(il):
                inst = il[i]
                si = inst.sync_info
                if si is not None and si.on_wait and len(si.on_wait) > maxw:
                    waits = list(si.on_wait)
                    keep = waits[-maxw:]
                    extra = waits[:-maxw]
                    inst.sync_info = mybir.SyncInfo(
                        on_wait=keep, on_update=list(si.on_update or [])
                    )
                    for w in extra:
                        b = nc.engines[inst.engine].nop(nofuse=True)
                        carrier = b.ins
                        popped = tail_bb.instructions.pop()
                        assert popped is carrier, "nop landed in unexpected block"
                        carrier.sync_info = mybir.SyncInfo(on_wait=[w], on_update=[])
                        il.insert(i, carrier)
                        i += 1
                        n += 1
                i += 1
    return n


def build_core_kernel(split_waits=True):
    nc = bass.Bass("TRN2", target_bir_lowering=False, debug=False, num_devices=1)

    def inp(name, shape, dt=F32):
        return nc.dram_tensor(name, shape, dt, kind="ExternalInput").ap()

    xqT = inp("xqT", [H, SQL], FP8)
    xkvT = inp("xkvT", [H, SKV], FP8)
    wqT = inp("wqT", [H, H], FP8)
    wkT = inp("wkT", [H, H], FP8)
    wvT = inp("wvT", [H, H], FP8)
    woT = inp("woT", [H, H], FP8)
    bqc = inp("bqc", [P, KC])
    bkc = inp("bkc", [P, KC])
    bvr = inp("bvr", [P, H])
    resid = inp("resid", [SQL, H])
    gam = inp("gam", [P, H])
    out = nc.dram_tensor("out", [SQL, H], F32, kind="ExternalOutput").ap()

    with SplitDrainTileContext(nc) as tc:
        with (
            nc.allow_low_precision(reason="fp8 attention at x64 scale"),
            tc.tile_pool(name="pers", bufs=1) as pers,
            tc.tile_pool(name="e2", bufs=3) as epool,
            tc.tile_pool(name="sm", bufs=6) as smpool,
            tc.tile_pool(name="rsd", bufs=3) as rspool,
            tc.tile_pool(name="hsb", bufs=2) as hpool,
            tc.tile_pool(name="sq", bufs=2) as sqpool,
            tc.tile_pool(name="tiny", bufs=6) as tpool,
            tc.tile_pool(name="psc", bufs=2, space="PSUM") as spool,
            tc.tile_pool(name="pcx", bufs=2, space="PSUM") as cpool,
            tc.tile_pool(name="pmi", bufs=2, space="PSUM") as mpool,
        ):
            # --- persistent tiles ------------------------------------------
            qt = pers.tile([P, KC, SQL], FP8, name="qt")          # 64*Q^T [o,s]
            ktall = pers.tile([P, KC, SKV], FP8, name="ktall")    # 64*K^T [o,s]
            v_sb = pers.tile([P, NKV, NH, HD + 1], FP8, name="v_sb")
            ctxT = pers.tile([P, KC, SQL], FP8, name="ctxT")      # 64*ctx^T
            accum_o = pers.tile([P, KC, H], BF16, name="accum_o")  # 4096*O
            wq_sb = pers.tile([P, KC, H], FP8, name="wq_sb")
            wk_sb = pers.tile([P, KC, H], FP8, name="wk_sb")
            wv_sb = pers.tile([P, KC, H], FP8, name="wv_sb")
            wo_sb = pers.tile([P, KC, H], FP8, name="wo_sb")
            xq_sb = pers.tile([P, KC, SQL], FP8, name="xq_sb")
            xkv_sb = pers.tile([P, KC, SKV], FP8, name="xkv_sb")
            bq_sb = pers.tile([P, KC], F32, name="bq_sb")
            bk_sb = pers.tile([P, KC], F32, name="bk_sb")
            bv_sb = pers.tile([P, H], F32, name="bv_sb")
            gam_sb = pers.tile([P, H], F32, name="gam_sb")
            ones64 = pers.tile([1, HD], BF16, name="ones64")
            eps_sb = pers.tile([P, 1], F32, name="eps_sb")
            nc.vector.memset(eps_sb, EPS_HAT)
            nc.vector.memset(ones64, WS)
            nc.vector.memset(v_sb[:, :, :, HD], WS)

            # --- DMAs (order matters for startup latency) ------------------
            nc.sync.dma_start(bq_sb, bqc)
            nc.sync.dma_start(bk_sb, bkc)
            nc.sync.dma_start(bv_sb, bvr)
            nc.sync.dma_start(gam_sb, gam)
            for ic in range(KC):
                nc.sync.dma_start(wq_sb[:, ic, :], wqT[ic * P : (ic + 1) * P, :])
            for ic in range(KC):
                nc.sync.dma_start(
                    xq_sb[:, ic, 0:512], xqT[ic * P : (ic + 1) * P, 0:512]
                )
            for ic in range(KC):
                nc.sync.dma_start(wk_sb[:, ic, :], wkT[ic * P : (ic + 1) * P, :])
            for ic in range(KC):
                nc.sync.dma_start(xkv_sb[:, ic, :], xkvT[ic * P : (ic + 1) * P, :])
            for ic in range(KC):
                nc.sync.dma_start(wv_sb[:, ic, :], wvT[ic * P : (ic + 1) * P, :])
            for ic in range(KC):
                nc.sync.dma_start(
                    xq_sb[:, ic, 512:1024], xqT[ic * P : (ic + 1) * P, 512:1024]
                )
            for ic in range(KC):
                nc.sync.dma_start(wo_sb[:, ic, :], woT[ic * P : (ic + 1) * P, :])

            # --- projection quanta (generators yielding per-MM) ------------
            def proj_q(oc, sqc):
                ps = mpool.tile([P, 512], F32, tag="pmi", name="ps_q")
                for t in range(KC // 2):
                    nc.tensor.matmul(
                        ps,
                        wq_sb[:, 2 * t : 2 * t + 2, oc * P : (oc + 1) * P],
                        xq_sb[:, 2 * t : 2 * t + 2, sqc * 512 : (sqc + 1) * 512],
                        start=(t == 0),
                        stop=(t == KC // 2 - 1),
                        perf_mode=DR,
                    )
                    yield
                nc.vector.tensor_scalar_add(
                    qt[:, oc, sqc * 512 : (sqc + 1) * 512], ps, bq_sb[:, oc : oc + 1]
                )
                yield

            def proj_k(oc, sc):
                ps = mpool.tile([P, 512], F32, tag="pmi", name="ps_k")
                for t in range(KC // 2):
                    nc.tensor.matmul(
                        ps,
                        wk_sb[:, 2 * t : 2 * t + 2, oc * P : (oc + 1) * P],
                        xkv_sb[:, 2 * t : 2 * t + 2, sc * 512 : (sc + 1) * 512],
                        start=(t == 0),
                        stop=(t == KC // 2 - 1),
                        perf_mode=DR,
                    )
                    yield
                nc.vector.tensor_scalar_add(
                    ktall[:, oc, sc * 512 : (sc + 1) * 512], ps, bk_sb[:, oc : oc + 1]
                )
                yield

            def proj_v(kvc, half):
                ps = mpool.tile([P, 512], F32, tag="pmi", name="ps_v")
                for t in range(KC // 2):
                    nc.tensor.matmul(
                        ps,
                        xkv_sb[:, 2 * t : 2 * t + 2, kvc * P : (kvc + 1) * P],
                        wv_sb[:, 2 * t : 2 * t + 2, half * 512 : (half + 1) * 512],
                        start=(t == 0),
                        stop=(t == KC // 2 - 1),
                        perf_mode=DR,
                    )
                    yield
                nc.vector.tensor_tensor(
                    v_sb[:, kvc, half * 8 : (half + 1) * 8, 0:HD],
                    ps.rearrange("p (h d) -> p h d", d=HD),
                    bv_sb[:, half * 512 : (half + 1) * 512].rearrange(
                        "p (h d) -> p h d", d=HD
                    ),
                    OP.add,
                )
                yield

            def proj_o_dr(t):
                # one DoubleRow pass over cc pair (2t, 2t+1), all 16 out tiles
                for s2 in range(KC):
                    for oc2 in range(2):
                        po = mpool.tile([P, 512], F32, tag="pmi", name="ps_o")
                        nc.tensor.matmul(
                            po,
                            ctxT[:, 2 * t : 2 * t + 2, s2 * P : (s2 + 1) * P],
                            wo_sb[:, 2 * t : 2 * t + 2, oc2 * 512 : (oc2 + 1) * 512],
                            start=True,
                            stop=True,
                            perf_mode=DR,
                        )
                        dst = accum_o[:, s2, oc2 * 512 : (oc2 + 1) * 512]
                        if t == 0:
                            nc.vector.tensor_copy(dst, po)
                        else:
                            nc.vector.tensor_tensor(dst, dst, po, OP.add)
                        yield

            def proj_o_single(cc):
                for s2 in range(KC):
                    for oc2 in range(2):
                        po = mpool.tile([P, 512], F32, tag="pmi", name="ps_o1")
                        nc.tensor.matmul(
                            po,
                            ctxT[:, cc, s2 * P : (s2 + 1) * P],
                            wo_sb[:, cc, oc2 * 512 : (oc2 + 1) * 512],
                            start=True,
                            stop=True,
                        )
                        dst = accum_o[:, s2, oc2 * 512 : (oc2 + 1) * 512]
                        nc.vector.tensor_tensor(dst, dst, po, OP.add)
                        yield

            def resid_dma(c):
                rs = rspool.tile([P, H], F32, tag="rsd", name="rs")
                nc.sync.dma_start(rs, resid[c * P : (c + 1) * P, :])
                resid_tiles[c] = rs
                yield

            resid_tiles = {}

            def chain(*gens):
                for g in gens:
                    yield from g

            def pads(n):
                for _ in range(n):
                    yield

            # --- filler schedule (iter = (hp, sqc), 16 kvc steps each) -----
            # budgets: iter0: 6/step, iter1: 4/step, else 3/step
            def iter_filler(it):
                if it == 0:
                    return chain(
                        proj_k(0, 1), proj_v(2, 0), proj_v(3, 0),
                        proj_k(0, 2), proj_v(4, 0), proj_v(5, 0),
                        proj_k(0, 3), proj_v(6, 0), proj_v(7, 0),
                        proj_v(8, 0), proj_v(9, 0), proj_q(0, 1),
                        proj_v(10, 0), proj_v(11, 0), proj_v(12, 0),
                        proj_v(13, 0), proj_v(14, 0), proj_v(15, 0),
                    )
                if it == 1:
                    return chain(
                        proj_k(1, 0), proj_k(1, 1), proj_k(1, 2), proj_k(1, 3),
                        proj_q(1, 0), proj_q(1, 1),
                        proj_v(0, 1), proj_v(1, 1), proj_v(2, 1), proj_v(3, 1),
                        proj_v(4, 1), proj_v(5, 1),
                    )
                if it == 2:
                    return chain(
                        proj_k(2, 0), proj_k(2, 1), proj_k(2, 2), proj_k(2, 3),
                        proj_q(2, 0), proj_q(2, 1),
                        proj_v(6, 1), proj_v(7, 1), proj_v(8, 1),
                    )
                if it == 3:
                    return chain(
                        proj_v(9, 1), proj_v(10, 1), proj_v(11, 1), proj_v(12, 1),
                        proj_v(13, 1), proj_v(14, 1), proj_v(15, 1),
                    )
                if it == 4:
                    return chain(
                        proj_k(3, 0), proj_k(3, 1), proj_k(3, 2), proj_k(3, 3),
                        proj_q(3, 0), proj_q(3, 1),
                    )
                if it == 5:
                    return proj_o_dr(0)
                if it == 6:
                    return chain(
                        proj_k(4, 0), proj_k(4, 1), proj_k(4, 2), proj_k(4, 3),
                        proj_q(4, 0), proj_q(4, 1),
                    )
                if it == 8:
                    return chain(
                        proj_k(5, 0), proj_k(5, 1), proj_k(5, 2), proj_k(5, 3),
                        proj_q(5, 0), proj_q(5, 1),
                    )
                if it == 9:
                    return proj_o_dr(1)
                if it == 10:
                    return chain(
                        proj_k(6, 0), proj_k(6, 1), proj_k(6, 2), proj_k(6, 3),
                        proj_q(6, 0), proj_q(6, 1),
                    )
                if it == 12:
                    return chain(
                        proj_k(7, 0), proj_k(7, 1), proj_k(7, 2), proj_k(7, 3),
                        proj_q(7, 0), proj_q(7, 1),
                    )
                if it == 13:
                    return proj_o_dr(2)
                if it == 14:
                    return chain(
                        pads(6),  # pair-6 sqc1 norms drain at step 1
                        proj_o_single(6),
                        resid_dma(0),
                    )
                if it == 15:
                    return chain(resid_dma(1), resid_dma(2))
                return pads(0)

            # --- prologue: just enough for pair 0 to start -----------------
            for _ in proj_q(0, 0):
                pass
            for _ in proj_k(0, 0):
                pass
            for _ in proj_v(0, 0):
                pass
            for _ in proj_v(1, 0):
                pass

            # --- attention main loop ---------------------------------------
            pending_norms = []

            def emit_pair(hp, sqc, it, filler, budget):
                cps = [
                    cpool.tile([P, 512], F32, tag="pcx", name="ps_ctx")
                    for _ in range(2)
                ]

                def emit_ctx(kt, e2):
                    for h in range(2):
                        nc.tensor.matmul(
                            cps[h][0 : HD + 1, :],
                            v_sb[:, 2 * kt : 2 * kt + 2, 2 * hp + h, :],
                            e2[:, :, h * 512 : (h + 1) * 512],
                            start=(kt == 0),
                            stop=(kt == NKT - 1),
                            perf_mode=DR,
                        )

                prev = None
                for kt in range(NKT):
                    e2 = epool.tile([P, 2, 1024], FP8, tag="e2", name="e2")
                    for j in range(2):
                        kvc = 2 * kt + j
                        if kvc == 1:
                            while pending_norms:
                                pending_norms.pop(0)()
                        sp = spool.tile([P, 1024], F32, tag="psc", name="ps_sc")
                        for h in range(2):
                            nc.tensor.matmul(
                                sp[:, h * 512 : (h + 1) * 512],
                                ktall[
                                    h * HD : (h + 1) * HD, hp, kvc * P : (kvc + 1) * P
                                ],
                                qt[
                                    h * HD : (h + 1) * HD,
                                    hp,
                                    sqc * 512 : (sqc + 1) * 512,
                                ],
                                start=True,
                                stop=True,
                            )
                        for _ in range(budget):
                            next(filler, None)
                        nc.scalar.activation(e2[:, j, :], sp, AF.Exp, scale=EXPS)
                    if prev is not None:
                        emit_ctx(kt - 1, prev)
                    prev = e2
                emit_ctx(NKT - 1, prev)

                # softmax denominators: 1/x as exp(-ln(x)) on ACT (a [1,512]
                # DVE reciprocal costs 3.4us; these cost 2x ~700ns on ACT)
                recs = []
                for h in range(2):
                    lnt = smpool.tile([1, 512], F32, tag="sm", name="lnt")
                    nc.scalar.activation(lnt, cps[h][HD : HD + 1, :], AF.Ln)
                    rec = smpool.tile([1, 512], BF16, tag="sm", name="rec")
                    nc.scalar.activation(rec, lnt, AF.Exp, scale=-1.0)
                    recs.append(rec)

                def deferred_norm(hp=hp, sqc=sqc, cps=cps, recs=recs):
                    for h in range(2):
                        rp = mpool.tile([P, 512], F32, tag="pmi", name="ps_r")
                        nc.tensor.matmul(
                            rp[0:HD, :], ones64, recs[h], start=True, stop=True
                        )
                        r_sb = smpool.tile([HD, 512], F32, tag="sm", name="r_sb")
                        nc.vector.tensor_copy(r_sb, rp[0:HD, :])
                        dst = ctxT[
                            h * HD : (h + 1) * HD, hp, sqc * 512 : (sqc + 1) * 512
                        ]
                        if h == 0:
                            nc.vector.tensor_tensor(
                                dst, cps[h][0:HD, :], r_sb, OP.mult
                            )
                        else:
                            stg = smpool.tile([HD, 512], FP8, tag="sm", name="stg")
                            nc.vector.tensor_tensor(
                                stg, cps[h][0:HD, :], r_sb, OP.mult
                            )
                            nc.sync.dma_start(dst, stg)

                pending_norms.append(deferred_norm)

            for it in range(16):
                hp, sqc = divmod(it, 2)
                budget = 6 if it == 0 else (4 if it == 1 else 3)
                filler = iter_filler(it)
                emit_pair(hp, sqc, it, filler, budget)
                for _ in filler:  # drain any leftovers
                    pass

            while pending_norms:
                pending_norms.pop(0)()
            for _ in proj_o_single(7):
                pass

            # --- RMSNorm epilogue (rs prefetch pipelined 3 deep) -----------
            for c in range(KC):
                if c + 3 < KC:
                    for _ in resid_dma(c + 3):
                        pass
                rs = resid_tiles[c]
                h_sb = hpool.tile([P, H], F32, tag="hsb", name="h_sb")
                nc.vector.tensor_tensor(h_sb, accum_o[:, c, :], rs, OP.add)
                sq = sqpool.tile([P, H], F32, tag="sq", name="sq_scr")
                ss = tpool.tile([P, 1], F32, tag="tiny", name="ss")
                nc.vector.tensor_tensor(sq, h_sb, h_sb, OP.mult)
                nc.vector.tensor_reduce(ss, sq, axis=mybir.AxisListType.X, op=OP.add)
                sr = tpool.tile([P, 1], F32, tag="tiny", name="sr")
                nc.scalar.activation(sr, ss, AF.Sqrt, scale=1.0 / H, bias=eps_sb)
                rr = tpool.tile([P, 1], F32, tag="tiny", name="rr")
                nc.vector.reciprocal(rr, sr)
                nc.vector.tensor_scalar_mul(h_sb, h_sb, rr)
                nc.gpsimd.tensor_tensor(rs, h_sb, gam_sb, OP.mult)
                nc.sync.dma_start(out[c * P : (c + 1) * P, :], rs)

    if split_waits:
        _split_sync_waits(nc)
    return nc


_NC = None


def _get_nc():
    global _NC
    if _NC is None:
        _NC = build_core_kernel()
    return _NC


def _fp8(a):
    return np.clip(a, -240.0, 240.0).astype(ml_dtypes.float8_e4m3)


def make_in_maps(hidden_states, keyvalue_states, Wq, bq, Wk, bk, Wv, bv, Wo, bo, gamma):
    f = np.float32
    hidden_states = np.asarray(hidden_states, f)
    keyvalue_states = np.asarray(keyvalue_states, f)
    shared = {
        "wqT": _fp8(np.asarray(Wq, f).T * WS),
        "wkT": _fp8(np.asarray(Wk, f).T * WS),
        "wvT": _fp8(np.asarray(Wv, f).T * WS),
        "woT": _fp8(np.asarray(Wo, f).T * WS),
        "bqc": np.ascontiguousarray(np.asarray(bq, f).reshape(KC, P).T * WS),
        "bkc": np.ascontiguousarray(np.asarray(bk, f).reshape(KC, P).T * WS),
        "bvr": np.ascontiguousarray(np.tile(np.asarray(bv, f) * WS, (P, 1))),
        "gam": np.ascontiguousarray(np.tile(np.asarray(gamma, f), (P, 1))),
    }
    bo = np.asarray(bo, f)
    in_maps = []
    for core in range(N_CORES):
        b, half = divmod(core, 2)
        hq = hidden_states[b, half * SQL : (half + 1) * SQL, :]
        m = dict(shared)
        m["xqT"] = _fp8(hq.T)
        m["xkvT"] = _fp8(keyvalue_states[b].T)
        m["resid"] = np.ascontiguousarray((hq + bo) * RS)
        in_maps.append(m)
    return in_maps


def _run(in_maps, trace=False, **kwargs):
    nc = _get_nc()
    return bass_utils.run_bass_kernel_spmd(
        nc, in_maps, core_ids=list(range(N_CORES)), trace=trace, **kwargs
    )


def _assemble(res):
    out = np.empty((B, SQ, H), np.float32)
    for core in range(N_CORES):
        b, half = divmod(core, 2)
        out[b, half * SQL : (half + 1) * SQL, :] = res.results[core]["out"]
    return out


def kernel(hidden_states, keyvalue_states, Wq, bq, Wk, bk, Wv, bv, Wo, bo, gamma):
    in_maps = make_in_maps(
        hidden_states, keyvalue_states, Wq, bq, Wk, bk, Wv, bv, Wo, bo, gamma
    )
    return _assemble(_run(in_maps))


# revision 18
# speedup vs baseline: 1.2270x; 1.0530x over previous
"""Trainium2 Bass kernel for a cross-attention layer (CoAttention + RMSNorm output).

Reference computation (per batch b):
    q = hidden @ Wq.T + bq ; k = kv @ Wk.T + bk ; v = kv @ Wv.T + bv
    probs = softmax(q k^T / sqrt(64))
    ctx = probs @ v
    out = RMSNorm(ctx @ Wo.T + bo + hidden) * gamma

Sharding: 8 cores = 4 batches x 2 query-row halves; no cross-core comms.

Numerics: everything upstream of the residual add runs in fp8(e4m3) with a
global x64 weight scale (W' = 64W). Scale bookkeeping:
  q' = x@Wq' = 64q (fp8), k' = 64k (fp8)   -> q'.k' = 4096 q.k
  exp scale 0.125/4096 recovers exp(q.k/8); e stored fp8 true-scale
  v' = 64v (fp8, ones column = 64)         -> ctx_psum = 64*ctx, row64 = 64*rowsum
  ctxT = ctx_psum * (64/row64) = 64*ctx_norm (fp8)
  O_psum = ctxT @ Wo' = 4096*O; resid is pre-scaled 4096(hidden+bo) on host
  RMSNorm(4096 h) == RMSNorm(h) with eps' = 4096^2 eps  (exact)

Engine plan (per core, target ~300us):
  PE  ~200us: fp8 DoubleRow (K=256/pass) projections Q/K/V/O + ctx; scores as
       two PE-row-tiled concurrent K=64 matmuls per kvc (216ns/pair warm).
       All projections stream as fillers inside the attention loop so the PE
       never idles (keeps the HAM clock-gate warm).
  ACT ~285us (pacer): one exp [128,1024] fp32 PSUM -> fp8 per kvc step.
  DVE: bias-adds, PSUM copybacks, softmax-denominator reciprocals, norms.
  GpSimd: gamma multiply in the RMSNorm epilogue.
"""

import numpy as np
import ml_dtypes

import concourse.bass as bass
import concourse.mybir as mybir
from concourse import bass_utils, tile

P = 128
H = 1024
NH = 16
HD = 64
B = 4
SQ = 2048
SQL = 1024  # per-core query rows
SKV = 2048
KC = H // P  # 8 contraction chunks of 128
NKV = SKV // P  # 16 kv chunks
NKT = NKV // 2  # kv chunk pairs (DoubleRow)
EPS = 1e-6

WS = 64.0  # weight scale
RS = WS * WS  # resid scale (4096)
EXPS = 0.125 / RS
EPS_HAT = EPS * RS * RS

F32 = mybir.dt.float32
BF16 = mybir.dt.bfloat16
FP8 = mybir.dt.float8e4
AF = mybir.ActivationFunctionType
OP = mybir.AluOpType
DR = mybir.MatmulPerfMode.DoubleRow

N_CORES = 8


class SplitDrainTileContext(tile.TileContext):
    """TileContext whose tail drain splits sem waits across chained drains.

    The walrus build in this container rejects CTRL instructions that carry
    more than one sync wait; the stock tail drain aggregates the whole global
    clock onto a single Drain instruction.
    """

    MAXW = 1

    def _drain_and_barrier(self, tick_clock, wait_clock):
        drain_inst = self.nc.sync.drain()
        wait_clock.add_sem_waits(
            drain_inst.ins, tile.ScopedClock({None: tick_clock.global_clock})
        )
        si = drain_inst.ins.sync_info
        if si is not None and si.on_wait and len(si.on_wait) > self.MAXW:
            waits = list(si.on_wait)
            drain_inst.ins.sync_info = mybir.SyncInfo(
                on_wait=waits[: self.MAXW], on_update=list(si.on_update or [])
            )
            rest = waits[self.MAXW :]
            for i in range(0, len(rest), self.MAXW):
                d2 = self.nc.sync.drain()
                d2.ins.sync_info = mybir.SyncInfo(
                    on_wait=rest[i : i + self.MAXW], on_update=[]
                )
        self.nc.all_engine_barrier()
        assert self.sems is not None
        popped = self.nc._tile_sem_poison_stack.pop()
        assert popped is self._sem_poison
        self.nc.clear_and_free_semaphores(list(self.sems.allocated().values()))
        self.nc.all_engine_barrier()


def _split_sync_waits(nc, maxw=1):
    """Hoist excess per-instruction sem waits onto preceding same-engine NoOps.

    The walrus build in this container rejects instructions carrying more
    than one sync wait command (any opcode family)."""
    n = 0
    tail_bb = nc.cur_bb.bb
    for f in nc.m.functions:
        for bb in f.blocks:
            il = bb.instructions
            i = 0
            while i < len(il):
                inst = il[i]
                si = inst.sync_info
                if si is not None and si.on_wait and len(si.on_wait) > maxw:
                    waits = list(si.on_wait)
                    keep = waits[-maxw:]
                    extra = waits[:-maxw]
                    inst.sync_info = mybir.SyncInfo(
                        on_wait=keep, on_update=list(si.on_update or [])
                    )
                    for w in extra:
                        b = nc.engines[inst.engine].nop(nofuse=True)
                        carrier = b.ins
                        popped = tail_bb.instructions.pop()
                        assert popped is carrier, "nop landed in unexpected block"
                        carrier.sync_info = mybir.SyncInfo(on_wait=[w], on_update=[])
                        il.insert(i, carrier)
                        i += 1
                        n += 1
                i += 1
    return n


def build_core_kernel(split_waits=True):
    nc = bass.Bass("TRN2", target_bir_lowering=False, debug=False, num_devices=1)

    def inp(name, shape, dt=F32):
        return nc.dram_tensor(name, shape, dt, kind="ExternalInput").ap()

    xqT = inp("xqT", [H, SQL], FP8)
    xkvT = inp("xkvT", [H, SKV], FP8)
    wqT = inp("wqT", [H, H], FP8)
    wkT = inp("wkT", [H, H], FP8)
    wvT = inp("wvT", [H, H], FP8)
    woT = inp("woT", [H, H], FP8)
    bqc = inp("bqc", [P, KC])
    bkc = inp("bkc", [P, KC])
    bvr = inp("bvr", [P, H])
    resid = inp("resid", [SQL, H])
    gam = inp("gam", [P, H])
    out = nc.dram_tensor("out", [SQL, H], F32, kind="ExternalOutput").ap()

    with SplitDrainTileContext(nc) as tc:
        with (
            nc.allow_low_precision(reason="fp8 attention at x64 scale"),
            tc.tile_pool(name="pers", bufs=1) as pers,
            tc.tile_pool(name="e2", bufs=3) as epool,
            tc.tile_pool(name="sm", bufs=6) as smpool,
            tc.tile_pool(name="rsd", bufs=3) as rspool,
            tc.tile_pool(name="hsb", bufs=2) as hpool,
            tc.tile_pool(name="sq", bufs=2) as sqpool,
            tc.tile_pool(name="tiny", bufs=6) as tpool,
            tc.tile_pool(name="psc", bufs=2, space="PSUM") as spool,
            tc.tile_pool(name="pcx", bufs=2, space="PSUM") as cpool,
            tc.tile_pool(name="pmi", bufs=2, space="PSUM") as mpool,
        ):
            # --- persistent tiles ------------------------------------------
            qt = pers.tile([P, KC, SQL], FP8, name="qt")          # 64*Q^T [o,s]
            ktall = pers.tile([P, KC, SKV], FP8, name="ktall")    # 64*K^T [o,s]
            v_sb = pers.tile([P, NKV, NH, HD + 1], FP8, name="v_sb")
            ctxT = pers.tile([P, KC, SQL], FP8, name="ctxT")      # 64*ctx^T
            accum_o = pers.tile([P, KC, H], BF16, name="accum_o")  # 4096*O
            wq_sb = pers.tile([P, KC, H], FP8, name="wq_sb")
            wk_sb = pers.tile([P, KC, H], FP8, name="wk_sb")
            wv_sb = pers.tile([P, KC, H], FP8, name="wv_sb")
            wo_sb = pers.tile([P, KC, H], FP8, name="wo_sb")
            xq_sb = pers.tile([P, KC, SQL], FP8, name="xq_sb")
            xkv_sb = pers.tile([P, KC, SKV], FP8, name="xkv_sb")
            bq_sb = pers.tile([P, KC], F32, name="bq_sb")
            bk_sb = pers.tile([P, KC], F32, name="bk_sb")
            bv_sb = pers.tile([P, H], F32, name="bv_sb")
            gam_sb = pers.tile([P, H], F32, name="gam_sb")
            ones64 = pers.tile([1, HD], BF16, name="ones64")
            eps_sb = pers.tile([P, 1], F32, name="eps_sb")
            nc.vector.memset(eps_sb, EPS_HAT)
            nc.vector.memset(ones64, WS)
            nc.vector.memset(v_sb[:, :, :, HD], WS)

            # --- DMAs (order matters for startup latency) ------------------
            nc.sync.dma_start(bq_sb, bqc)
            nc.sync.dma_start(bk_sb, bkc)
            nc.sync.dma_start(bv_sb, bvr)
            nc.sync.dma_start(gam_sb, gam)
            for ic in range(KC):
                nc.sync.dma_start(wq_sb[:, ic, :], wqT[ic * P : (ic + 1) * P, :])
            for ic in range(KC):
                nc.sync.dma_start(
                    xq_sb[:, ic, 0:512], xqT[ic * P : (ic + 1) * P, 0:512]
                )
            for ic in range(KC):
                nc.sync.dma_start(wk_sb[:, ic, :], wkT[ic * P : (ic + 1) * P, :])
            for ic in range(KC):
                nc.sync.dma_start(
                    xkv_sb[:, ic, 0:512], xkvT[ic * P : (ic + 1) * P, 0:512]
                )
            for ic in range(KC):
                nc.sync.dma_start(wv_sb[:, ic, :], wvT[ic * P : (ic + 1) * P, :])
            for ic in range(KC):
                nc.sync.dma_start(
                    xkv_sb[:, ic, 512:SKV], xkvT[ic * P : (ic + 1) * P, 512:SKV]
                )
            for ic in range(KC):
                nc.sync.dma_start(
                    xq_sb[:, ic, 512:1024], xqT[ic * P : (ic + 1) * P, 512:1024]
                )
            for ic in range(KC):
                nc.sync.dma_start(wo_sb[:, ic, :], woT[ic * P : (ic + 1) * P, :])

            # --- projection quanta (generators yielding per-MM) ------------
            def proj_q(oc, sqc):
                ps = mpool.tile([P, 512], F32, tag="pmi", name="ps_q")
                for t in range(KC // 2):
                    nc.tensor.matmul(
                        ps,
                        wq_sb[:, 2 * t : 2 * t + 2, oc * P : (oc + 1) * P],
                        xq_sb[:, 2 * t : 2 * t + 2, sqc * 512 : (sqc + 1) * 512],
                        start=(t == 0),
                        stop=(t == KC // 2 - 1),
                        perf_mode=DR,
                    )
                    yield
                nc.vector.tensor_scalar_add(
                    qt[:, oc, sqc * 512 : (sqc + 1) * 512], ps, bq_sb[:, oc : oc + 1]
                )
                yield

            def proj_k(oc, sc):
                ps = mpool.tile([P, 512], F32, tag="pmi", name="ps_k")
                for t in range(KC // 2):
                    nc.tensor.matmul(
                        ps,
                        wk_sb[:, 2 * t : 2 * t + 2, oc * P : (oc + 1) * P],
                        xkv_sb[:, 2 * t : 2 * t + 2, sc * 512 : (sc + 1) * 512],
                        start=(t == 0),
                        stop=(t == KC // 2 - 1),
                        perf_mode=DR,
                    )
                    yield
                nc.vector.tensor_scalar_add(
                    ktall[:, oc, sc * 512 : (sc + 1) * 512], ps, bk_sb[:, oc : oc + 1]
                )
                yield

            def proj_v(kvc, half):
                ps = mpool.tile([P, 512], F32, tag="pmi", name="ps_v")
                for t in range(KC // 2):
                    nc.tensor.matmul(
                        ps,
                        xkv_sb[:, 2 * t : 2 * t + 2, kvc * P : (kvc + 1) * P],
                        wv_sb[:, 2 * t : 2 * t + 2, half * 512 : (half + 1) * 512],
                        start=(t == 0),
                        stop=(t == KC // 2 - 1),
                        perf_mode=DR,
                    )
                    yield
                nc.vector.tensor_tensor(
                    v_sb[:, kvc, half * 8 : (half + 1) * 8, 0:HD],
                    ps.rearrange("p (h d) -> p h d", d=HD),
                    bv_sb[:, half * 512 : (half + 1) * 512].rearrange(
                        "p (h d) -> p h d", d=HD
                    ),
                    OP.add,
                )
                yield

            def proj_o_dr(t):
                # one DoubleRow pass over cc pair (2t, 2t+1), all 16 out tiles
                for s2 in range(KC):
                    for oc2 in range(2):
                        po = mpool.tile([P, 512], F32, tag="pmi", name="ps_o")
                        nc.tensor.matmul(
                            po,
                            ctxT[:, 2 * t : 2 * t + 2, s2 * P : (s2 + 1) * P],
                            wo_sb[:, 2 * t : 2 * t + 2, oc2 * 512 : (oc2 + 1) * 512],
                            start=True,
                            stop=True,
                            perf_mode=DR,
                        )
                        dst = accum_o[:, s2, oc2 * 512 : (oc2 + 1) * 512]
                        if t == 0:
                            nc.vector.tensor_copy(dst, po)
                        else:
                            nc.vector.tensor_tensor(dst, dst, po, OP.add)
                        yield

            def proj_o_single(cc, s2_range=None):
                for s2 in s2_range if s2_range is not None else range(KC):
                    for oc2 in range(2):
                        po = mpool.tile([P, 512], F32, tag="pmi", name="ps_o1")
                        nc.tensor.matmul(
                            po,
                            ctxT[:, cc, s2 * P : (s2 + 1) * P],
                            wo_sb[:, cc, oc2 * 512 : (oc2 + 1) * 512],
                            start=True,
                            stop=True,
                        )
                        dst = accum_o[:, s2, oc2 * 512 : (oc2 + 1) * 512]
                        nc.vector.tensor_tensor(dst, dst, po, OP.add)
                        yield

            def resid_dma(c):
                rs = rspool.tile([P, H], F32, tag="rsd", name="rs")
                nc.sync.dma_start(rs, resid[c * P : (c + 1) * P, :])
                resid_tiles[c] = rs
                yield

            resid_tiles = {}

            def chain(*gens):
                for g in gens:
                    yield from g

            def pads(n):
                for _ in range(n):
                    yield

            # --- filler schedule (iter = (hp, sqc), 16 kvc steps each) -----
            # budgets: iter0: 6/step, iter1: 4/step, else 3/step
            def iter_filler(it):
                if it == 0:
                    return chain(
                        proj_v(0, 0), proj_v(1, 0),
                        proj_k(0, 1), proj_v(2, 0), proj_v(3, 0),
                        proj_k(0, 2), proj_v(4, 0), proj_v(5, 0),
                        proj_k(0, 3), proj_v(6, 0), proj_v(7, 0),
                        proj_v(8, 0), proj_v(9, 0), proj_q(0, 1),
                        proj_v(10, 0), proj_v(11, 0), proj_v(12, 0),
                        proj_v(13, 0), proj_v(14, 0), proj_v(15, 0),
                    )
                if it == 1:
                    return chain(
                        proj_k(1, 0), proj_k(1, 1), proj_k(1, 2), proj_k(1, 3),
                        proj_q(1, 0), proj_q(1, 1),
                        proj_v(0, 1), proj_v(1, 1), proj_v(2, 1), proj_v(3, 1),
                        proj_v(4, 1), proj_v(5, 1),
                    )
                if it == 2:
                    return chain(
                        proj_k(2, 0), proj_k(2, 1), proj_k(2, 2), proj_k(2, 3),
                        proj_q(2, 0), proj_q(2, 1),
                        proj_v(6, 1), proj_v(7, 1), proj_v(8, 1),
                    )
                if it == 3:
                    return chain(
                        proj_v(9, 1), proj_v(10, 1), proj_v(11, 1), proj_v(12, 1),
                        proj_v(13, 1), proj_v(14, 1), proj_v(15, 1),
                    )
                if it == 4:
                    return chain(
                        proj_k(3, 0), proj_k(3, 1), proj_k(3, 2), proj_k(3, 3),
                        proj_q(3, 0), proj_q(3, 1),
                    )
                if it == 5:
                    return proj_o_dr(0)
                if it == 6:
                    return chain(
                        proj_k(4, 0), proj_k(4, 1), proj_k(4, 2), proj_k(4, 3),
                        proj_q(4, 0), proj_q(4, 1),
                    )
                if it == 8:
                    return chain(
                        proj_k(5, 0), proj_k(5, 1), proj_k(5, 2), proj_k(5, 3),
                        proj_q(5, 0), proj_q(5, 1),
                    )
                if it == 9:
                    return proj_o_dr(1)
                if it == 10:
                    return chain(
                        proj_k(6, 0), proj_k(6, 1), proj_k(6, 2), proj_k(6, 3),
                        proj_q(6, 0), proj_q(6, 1),
                    )
                if it == 12:
                    return chain(
                        proj_k(7, 0), proj_k(7, 1), proj_k(7, 2), proj_k(7, 3),
                        proj_q(7, 0), proj_q(7, 1),
                    )
                if it == 13:
                    return proj_o_dr(2)
                if it == 14:
                    return chain(
                        pads(9),  # pair-6 sqc1 norms finish at step 2
                        proj_o_single(6),
                        resid_dma(0),
                    )
                if it == 15:
                    return chain(
                        pads(12),  # pair-7 sqc0 norms finish at step 2
                        proj_o_single(7, s2_range=range(4)),
                        resid_dma(1), resid_dma(2),
                    )
                return pads(0)

            # --- prologue: just enough for pair 0 to start -----------------
            for _ in proj_q(0, 0):
                pass
            for _ in proj_k(0, 0):
                pass

            # --- attention main loop ---------------------------------------
            pending_a = []  # last-ctx + softmax-denominator recs
            pending_b = []  # R broadcast matmuls + ctxT normalize

            def emit_pair(hp, sqc, it, filler, budget):
                cps = [
                    cpool.tile([P, 512], F32, tag="pcx", name="ps_ctx")
                    for _ in range(2)
                ]

                def emit_ctx(kt, e2):
                    for h in range(2):
                        nc.tensor.matmul(
                            cps[h][0 : HD + 1, :],
                            v_sb[:, 2 * kt : 2 * kt + 2, 2 * hp + h, :],
                            e2[:, :, h * 512 : (h + 1) * 512],
                            start=(kt == 0),
                            stop=(kt == NKT - 1),
                            perf_mode=DR,
                        )

                prev = None
                for kt in range(NKT):
                    e2 = epool.tile([P, 2, 1024], FP8, tag="e2", name="e2")
                    for j in range(2):
                        kvc = 2 * kt + j
                        if kvc == 1:
                            while pending_a:
                                pending_a.pop(0)()
                        elif kvc == 2:
                            while pending_b:
                                pending_b.pop(0)()
                        sp = spool.tile([P, 1024], F32, tag="psc", name="ps_sc")
                        for h in range(2):
                            nc.tensor.matmul(
                                sp[:, h * 512 : (h + 1) * 512],
                                ktall[
                                    h * HD : (h + 1) * HD, hp, kvc * P : (kvc + 1) * P
                                ],
                                qt[
                                    h * HD : (h + 1) * HD,
                                    hp,
                                    sqc * 512 : (sqc + 1) * 512,
                                ],
                                start=True,
                                stop=True,
                            )
                        for _ in range(budget):
                            next(filler, None)
                        nc.scalar.activation(e2[:, j, :], sp, AF.Exp, scale=EXPS)
                    if prev is not None:
                        emit_ctx(kt - 1, prev)
                    prev = e2
                last_e2 = prev

                def deferred_tail(emit_ctx=emit_ctx, cps=cps, last_e2=last_e2):
                    # final ctx pair + softmax denominators: 1/x as exp(-ln(x))
                    # on ACT (a [1,512] DVE reciprocal costs 3.4us)
                    emit_ctx(NKT - 1, last_e2)
                    recs = []
                    for h in range(2):
                        lnt = smpool.tile([1, 512], F32, tag="sm", name="lnt")
                        nc.scalar.activation(lnt, cps[h][HD : HD + 1, :], AF.Ln)
                        rec = smpool.tile([1, 512], BF16, tag="sm", name="rec")
                        nc.scalar.activation(rec, lnt, AF.Exp, scale=-1.0)
                        recs.append(rec)

                    def deferred_norm(hp=hp, sqc=sqc, cps=cps, recs=recs):
                        for h in range(2):
                            rp = mpool.tile([P, 512], F32, tag="pmi", name="ps_r")
                            nc.tensor.matmul(
                                rp[0:HD, :], ones64, recs[h], start=True, stop=True
                            )
                            r_sb = smpool.tile([HD, 512], F32, tag="sm", name="r_sb")
                            nc.vector.tensor_copy(r_sb, rp[0:HD, :])
                            dst = ctxT[
                                h * HD : (h + 1) * HD, hp, sqc * 512 : (sqc + 1) * 512
                            ]
                            if h == 0:
                                nc.vector.tensor_tensor(
                                    dst, cps[h][0:HD, :], r_sb, OP.mult
                                )
                            else:
                                stg = smpool.tile([HD, 512], FP8, tag="sm", name="stg")
                                nc.vector.tensor_tensor(
                                    stg, cps[h][0:HD, :], r_sb, OP.mult
                                )
                                nc.sync.dma_start(dst, stg)

                    pending_b.append(deferred_norm)

                pending_a.append(deferred_tail)

            for it in range(16):
                hp, sqc = divmod(it, 2)
                budget = 6 if it == 0 else (4 if it == 1 else 3)
                filler = iter_filler(it)
                emit_pair(hp, sqc, it, filler, budget)
                for _ in filler:  # drain any leftovers
                    pass

            while pending_a:
                pending_a.pop(0)()
            while pending_b:
                pending_b.pop(0)()
            for _ in proj_o_single(7, s2_range=range(4, KC)):
                pass

            # --- RMSNorm epilogue (rs prefetch pipelined 3 deep) -----------
            for c in range(KC):
                if c + 3 < KC:
                    for _ in resid_dma(c + 3):
                        pass
                rs = resid_tiles[c]
                h_sb = hpool.tile([P, H], F32, tag="hsb", name="h_sb")
                nc.gpsimd.tensor_tensor(h_sb, accum_o[:, c, :], rs, OP.add)
                sq = sqpool.tile([P, H], F32, tag="sq", name="sq_scr")
                ss = tpool.tile([P, 1], F32, tag="tiny", name="ss")
                nc.vector.tensor_tensor(sq, h_sb, h_sb, OP.mult)
                nc.vector.tensor_reduce(ss, sq, axis=mybir.AxisListType.X, op=OP.add)
                sr = tpool.tile([P, 1], F32, tag="tiny", name="sr")
                nc.scalar.activation(sr, ss, AF.Sqrt, scale=1.0 / H, bias=eps_sb)
                rr = tpool.tile([P, 1], F32, tag="tiny", name="rr")
                nc.vector.reciprocal(rr, sr)
                nc.vector.tensor_scalar_mul(h_sb, h_sb, rr)
                nc.vector.tensor_tensor(rs, h_sb, gam_sb, OP.mult)
                nc.sync.dma_start(out[c * P : (c + 1) * P, :], rs)

    if split_waits:
        _split_sync_waits(nc)
    return nc


_NC = None


def _get_nc():
    global _NC
    if _NC is None:
        _NC = build_core_kernel()
    return _NC


def _fp8(a):
    return np.clip(a, -240.0, 240.0).astype(ml_dtypes.float8_e4m3)


def make_in_maps(hidden_states, keyvalue_states, Wq, bq, Wk, bk, Wv, bv, Wo, bo, gamma):
    f = np.float32
    hidden_states = np.asarray(hidden_states, f)
    keyvalue_states = np.asarray(keyvalue_states, f)
    shared = {
        "wqT": _fp8(np.asarray(Wq, f).T * WS),
        "wkT": _fp8(np.asarray(Wk, f).T * WS),
        "wvT": _fp8(np.asarray(Wv, f).T * WS),
        "woT": _fp8(np.asarray(Wo, f).T * WS),
        "bqc": np.ascontiguousarray(np.asarray(bq, f).reshape(KC, P).T * WS),
        "bkc": np.ascontiguousarray(np.asarray(bk, f).reshape(KC, P).T * WS),
        "bvr": np.ascontiguousarray(np.tile(np.asarray(bv, f) * WS, (P, 1))),
        "gam": np.ascontiguousarray(np.tile(np.asarray(gamma, f), (P, 1))),
    }
    bo = np.asarray(bo, f)
    in_maps = []
    for core in range(N_CORES):
        b, half = divmod(core, 2)
        hq = hidden_states[b, half * SQL : (half + 1) * SQL, :]
        m = dict(shared)
        m["xqT"] = _fp8(hq.T)
        m["xkvT"] = _fp8(keyvalue_states[b].T)
        m["resid"] = np.ascontiguousarray((hq + bo) * RS)
        in_maps.append(m)
    return in_maps


def _run(in_maps, trace=False, **kwargs):
    nc = _get_nc()
    return bass_utils.run_bass_kernel_spmd(
        nc, in_maps, core_ids=list(range(N_CORES)), trace=trace, **kwargs
    )


def _assemble(res):
    out = np.empty((B, SQ, H), np.float32)
    for core in range(N_CORES):
        b, half = divmod(core, 2)
        out[b, half * SQL : (half + 1) * SQL, :] = res.results[core]["out"]
    return out


def kernel(hidden_states, keyvalue_states, Wq, bq, Wk, bk, Wv, bv, Wo, bo, gamma):
    in_maps = make_in_maps(
        hidden_states, keyvalue_states, Wq, bq, Wk, bk, Wv, bv, Wo, bo, gamma
    )
    return _assemble(_run(in_maps))


# revision 23
# speedup vs baseline: 1.2282x; 1.0010x over previous
"""Trainium2 Bass kernel for a cross-attention layer (CoAttention + RMSNorm output).

Reference computation (per batch b):
    q = hidden @ Wq.T + bq ; k = kv @ Wk.T + bk ; v = kv @ Wv.T + bv
    probs = softmax(q k^T / sqrt(64))
    ctx = probs @ v
    out = RMSNorm(ctx @ Wo.T + bo + hidden) * gamma

Sharding: 8 cores = 4 batches x 2 query-row halves; no cross-core comms.

Numerics: everything upstream of the residual add runs in fp8(e4m3) with a
global x64 weight scale (W' = 64W). Scale bookkeeping:
  q' = x@Wq' = 64q (fp8), k' = 64k (fp8)   -> q'.k' = 4096 q.k
  exp scale 0.125/4096 recovers exp(q.k/8); e stored fp8 true-scale
  v' = 64v (fp8, ones column = 64)         -> ctx_psum = 64*ctx, row64 = 64*rowsum
  ctxT = ctx_psum * (64/row64) = 64*ctx_norm (fp8)
  O_psum = ctxT @ Wo' = 4096*O; resid is pre-scaled 4096(hidden+bo) on host
  RMSNorm(4096 h) == RMSNorm(h) with eps' = 4096^2 eps  (exact)

Engine plan (per core, target ~300us):
  PE  ~200us: fp8 DoubleRow (K=256/pass) projections Q/K/V/O + ctx; scores as
       two PE-row-tiled concurrent K=64 matmuls per kvc (216ns/pair warm).
       All projections stream as fillers inside the attention loop so the PE
       never idles (keeps the HAM clock-gate warm).
  ACT ~285us (pacer): one exp [128,1024] fp32 PSUM -> fp8 per kvc step.
  DVE: bias-adds, PSUM copybacks, softmax-denominator reciprocals, norms.
  GpSimd: gamma multiply in the RMSNorm epilogue.
"""

import numpy as np
import ml_dtypes

import concourse.bass as bass
import concourse.mybir as mybir
from concourse import bass_utils, tile

P = 128
H = 1024
NH = 16
HD = 64
B = 4
SQ = 2048
SQL = 1024  # per-core query rows
SKV = 2048
KC = H // P  # 8 contraction chunks of 128
NKV = SKV // P  # 16 kv chunks
NKT = NKV // 2  # kv chunk pairs (DoubleRow)
EPS = 1e-6

WS = 64.0  # weight scale
RS = WS * WS  # resid scale (4096)
EXPS = 0.125 / RS
EPS_HAT = EPS * RS * RS

F32 = mybir.dt.float32
BF16 = mybir.dt.bfloat16
FP8 = mybir.dt.float8e4
AF = mybir.ActivationFunctionType
OP = mybir.AluOpType
DR = mybir.MatmulPerfMode.DoubleRow

N_CORES = 8


class SplitDrainTileContext(tile.TileContext):
    """TileContext whose tail drain splits sem waits across chained drains.

    The walrus build in this container rejects CTRL instructions that carry
    more than one sync wait; the stock tail drain aggregates the whole global
    clock onto a single Drain instruction.
    """

    MAXW = 1

    def _drain_and_barrier(self, tick_clock, wait_clock):
        drain_inst = self.nc.sync.drain()
        wait_clock.add_sem_waits(
            drain_inst.ins, tile.ScopedClock({None: tick_clock.global_clock})
        )
        si = drain_inst.ins.sync_info
        if si is not None and si.on_wait and len(si.on_wait) > self.MAXW:
            waits = list(si.on_wait)
            drain_inst.ins.sync_info = mybir.SyncInfo(
                on_wait=waits[: self.MAXW], on_update=list(si.on_update or [])
            )
            rest = waits[self.MAXW :]
            for i in range(0, len(rest), self.MAXW):
                d2 = self.nc.sync.drain()
                d2.ins.sync_info = mybir.SyncInfo(
                    on_wait=rest[i : i + self.MAXW], on_update=[]
                )
        self.nc.all_engine_barrier()
        assert self.sems is not None
        popped = self.nc._tile_sem_poison_stack.pop()
        assert popped is self._sem_poison
        self.nc.clear_and_free_semaphores(list(self.sems.allocated().values()))
        self.nc.all_engine_barrier()


def _split_sync_waits(nc, maxw=1):
    """Hoist excess per-instruction sem waits onto preceding same-engine NoOps.

    The walrus build in this container rejects instructions carrying more
    than one sync wait command (any opcode family)."""
    n = 0
    tail_bb = nc.cur_bb.bb
    for f in nc.m.functions:
        for bb in f.blocks:
            il = bb.instructions
            i = 0
            while i < len(il):
                inst = il[i]
                si = inst.sync_info
                if si is not None and si.on_wait and len(si.on_wait) > maxw:
                    waits = list(si.on_wait)
                    keep = waits[-maxw:]
                    extra = waits[:-maxw]
                    inst.sync_info = mybir.SyncInfo(
                        on_wait=keep, on_update=list(si.on_update or [])
                    )
                    for w in extra:
                        b = nc.engines[inst.engine].nop(nofuse=True)
                        carrier = b.ins
                        popped = tail_bb.instructions.pop()
                        assert popped is carrier, "nop landed in unexpected block"
                        carrier.sync_info = mybir.SyncInfo(on_wait=[w], on_update=[])
                        il.insert(i, carrier)
                        i += 1
                        n += 1
                i += 1
    return n


def build_core_kernel(split_waits=True):
    nc = bass.Bass("TRN2", target_bir_lowering=False, debug=False, num_devices=1)

    def inp(name, shape, dt=F32):
        return nc.dram_tensor(name, shape, dt, kind="ExternalInput").ap()

    xqT = inp("xqT", [H, SQL], FP8)
    xkvT = inp("xkvT", [H, SKV], FP8)
    wqT = inp("wqT", [H, H], FP8)
    wkT = inp("wkT", [H, H], FP8)
    wvT = inp("wvT", [H, H], FP8)
    woT = inp("woT", [H, H], FP8)
    bqc = inp("bqc", [P, KC])
    bkc = inp("bkc", [P, KC])
    bvr = inp("bvr", [P, H])
    resid = inp("resid", [SQL, H])
    gam = inp("gam", [P, H])
    out = nc.dram_tensor("out", [SQL, H], F32, kind="ExternalOutput").ap()

    with SplitDrainTileContext(nc) as tc:
        with (
            nc.allow_low_precision(reason="fp8 attention at x64 scale"),
            tc.tile_pool(name="pers", bufs=1) as pers,
            tc.tile_pool(name="e2", bufs=3) as epool,
            tc.tile_pool(name="sm", bufs=6) as smpool,
            tc.tile_pool(name="rsd", bufs=3) as rspool,
            tc.tile_pool(name="hsb", bufs=2) as hpool,
            tc.tile_pool(name="sq", bufs=2) as sqpool,
            tc.tile_pool(name="tiny", bufs=6) as tpool,
            tc.tile_pool(name="psc", bufs=2, space="PSUM") as spool,
            tc.tile_pool(name="pcx", bufs=2, space="PSUM") as cpool,
            tc.tile_pool(name="pmi", bufs=2, space="PSUM") as mpool,
        ):
            # --- persistent tiles ------------------------------------------
            qt = pers.tile([P, KC, SQL], FP8, name="qt")          # 64*Q^T [o,s]
            ktall = pers.tile([P, KC, SKV], FP8, name="ktall")    # 64*K^T [o,s]
            v_sb = pers.tile([P, NKV, NH, HD + 1], FP8, name="v_sb")
            ctxT = pers.tile([P, KC, SQL], FP8, name="ctxT")      # 64*ctx^T
            accum_o = pers.tile([P, KC, H], BF16, name="accum_o")  # 4096*O
            wq_sb = pers.tile([P, KC, H], FP8, name="wq_sb")
            wk_sb = pers.tile([P, KC, H], FP8, name="wk_sb")
            wv_sb = pers.tile([P, KC, H], FP8, name="wv_sb")
            wo_sb = pers.tile([P, KC, H], FP8, name="wo_sb")
            xq_sb = pers.tile([P, KC, SQL], FP8, name="xq_sb")
            xkv_sb = pers.tile([P, KC, SKV], FP8, name="xkv_sb")
            bq_sb = pers.tile([P, KC], F32, name="bq_sb")
            bk_sb = pers.tile([P, KC], F32, name="bk_sb")
            bv_sb = pers.tile([P, H], F32, name="bv_sb")
            gam_sb = pers.tile([P, H], F32, name="gam_sb")
            ones64 = pers.tile([1, HD], BF16, name="ones64")
            eps_sb = pers.tile([P, 1], F32, name="eps_sb")
            nc.vector.memset(eps_sb, EPS_HAT)
            nc.vector.memset(ones64, WS)
            nc.vector.memset(v_sb[:, :, :, HD], WS)

            # --- DMAs (order matters for startup latency) ------------------
            nc.sync.dma_start(bq_sb, bqc)
            nc.sync.dma_start(bk_sb, bkc)
            nc.sync.dma_start(bv_sb, bvr)
            nc.sync.dma_start(gam_sb, gam)
            for ic in range(KC):
                nc.sync.dma_start(wq_sb[:, ic, :], wqT[ic * P : (ic + 1) * P, :])
            for ic in range(KC):
                nc.sync.dma_start(
                    xq_sb[:, ic, 0:512], xqT[ic * P : (ic + 1) * P, 0:512]
                )
            for ic in range(KC):
                nc.sync.dma_start(wk_sb[:, ic, :], wkT[ic * P : (ic + 1) * P, :])
            for ic in range(KC):
                nc.sync.dma_start(
                    xkv_sb[:, ic, 0:512], xkvT[ic * P : (ic + 1) * P, 0:512]
                )
            for ic in range(KC):
                nc.sync.dma_start(
                    wv_sb[:, ic, 0:512], wvT[ic * P : (ic + 1) * P, 0:512]
                )
            for ic in range(KC):
                nc.sync.dma_start(
                    xkv_sb[:, ic, 512:SKV], xkvT[ic * P : (ic + 1) * P, 512:SKV]
                )
            for ic in range(KC):
                nc.sync.dma_start(
                    wv_sb[:, ic, 512:H], wvT[ic * P : (ic + 1) * P, 512:H]
                )
            for ic in range(KC):
                nc.sync.dma_start(
                    xq_sb[:, ic, 512:1024], xqT[ic * P : (ic + 1) * P, 512:1024]
                )
            for ic in range(KC):
                nc.sync.dma_start(wo_sb[:, ic, :], woT[ic * P : (ic + 1) * P, :])

            # --- projection quanta (generators yielding per-MM) ------------
            def proj_q(oc, sqc):
                ps = mpool.tile([P, 512], F32, tag="pmi", name="ps_q")
                for t in range(KC // 2):
                    nc.tensor.matmul(
                        ps,
                        wq_sb[:, 2 * t : 2 * t + 2, oc * P : (oc + 1) * P],
                        xq_sb[:, 2 * t : 2 * t + 2, sqc * 512 : (sqc + 1) * 512],
                        start=(t == 0),
                        stop=(t == KC // 2 - 1),
                        perf_mode=DR,
                    )
                    yield
                nc.vector.tensor_scalar_add(
                    qt[:, oc, sqc * 512 : (sqc + 1) * 512], ps, bq_sb[:, oc : oc + 1]
                )
                yield

            def proj_k(oc, sc):
                ps = mpool.tile([P, 512], F32, tag="pmi", name="ps_k")
                for t in range(KC // 2):
                    nc.tensor.matmul(
                        ps,
                        wk_sb[:, 2 * t : 2 * t + 2, oc * P : (oc + 1) * P],
                        xkv_sb[:, 2 * t : 2 * t + 2, sc * 512 : (sc + 1) * 512],
                        start=(t == 0),
                        stop=(t == KC // 2 - 1),
                        perf_mode=DR,
                    )
                    yield
                nc.vector.tensor_scalar_add(
                    ktall[:, oc, sc * 512 : (sc + 1) * 512], ps, bk_sb[:, oc : oc + 1]
                )
                yield

            def proj_v(kvc, half):
                ps = mpool.tile([P, 512], F32, tag="pmi", name="ps_v")
                for t in range(KC // 2):
                    nc.tensor.matmul(
                        ps,
                        xkv_sb[:, 2 * t : 2 * t + 2, kvc * P : (kvc + 1) * P],
                        wv_sb[:, 2 * t : 2 * t + 2, half * 512 : (half + 1) * 512],
                        start=(t == 0),
                        stop=(t == KC // 2 - 1),
                        perf_mode=DR,
                    )
                    yield
                nc.vector.tensor_tensor(
                    v_sb[:, kvc, half * 8 : (half + 1) * 8, 0:HD],
                    ps.rearrange("p (h d) -> p h d", d=HD),
                    bv_sb[:, half * 512 : (half + 1) * 512].rearrange(
                        "p (h d) -> p h d", d=HD
                    ),
                    OP.add,
                )
                yield

            def proj_o_dr(t):
                # one DoubleRow pass over cc pair (2t, 2t+1), all 16 out tiles
                for s2 in range(KC):
                    for oc2 in range(2):
                        po = mpool.tile([P, 512], F32, tag="pmi", name="ps_o")
                        nc.tensor.matmul(
                            po,
                            ctxT[:, 2 * t : 2 * t + 2, s2 * P : (s2 + 1) * P],
                            wo_sb[:, 2 * t : 2 * t + 2, oc2 * 512 : (oc2 + 1) * 512],
                            start=True,
                            stop=True,
                            perf_mode=DR,
                        )
                        dst = accum_o[:, s2, oc2 * 512 : (oc2 + 1) * 512]
                        if t == 0:
                            nc.vector.tensor_copy(dst, po)
                        else:
                            nc.vector.tensor_tensor(dst, dst, po, OP.add)
                        yield

            def proj_o_single(cc, s2_range=None):
                for s2 in s2_range if s2_range is not None else range(KC):
                    for oc2 in range(2):
                        po = mpool.tile([P, 512], F32, tag="pmi", name="ps_o1")
                        nc.tensor.matmul(
                            po,
                            ctxT[:, cc, s2 * P : (s2 + 1) * P],
                            wo_sb[:, cc, oc2 * 512 : (oc2 + 1) * 512],
                            start=True,
                            stop=True,
                        )
                        dst = accum_o[:, s2, oc2 * 512 : (oc2 + 1) * 512]
                        nc.vector.tensor_tensor(dst, dst, po, OP.add)
                        yield

            def resid_dma(c):
                rs = rspool.tile([P, H], F32, tag="rsd", name="rs")
                nc.sync.dma_start(rs, resid[c * P : (c + 1) * P, :])
                resid_tiles[c] = rs
                yield

            resid_tiles = {}

            def chain(*gens):
                for g in gens:
                    yield from g

            def pads(n):
                for _ in range(n):
                    yield

            # --- filler schedule (iter = (hp, sqc), 16 kvc steps each) -----
            # quanta are fed into ONE persistent queue; each iter appends its
            # content and the loop consumes `budget` per kvc step so the PE
            # load stays even (HAM clock-gate must never see an idle window).
            def qk_next(oc):
                return chain(
                    proj_k(oc, 0), proj_k(oc, 1), proj_k(oc, 2), proj_k(oc, 3),
                    proj_q(oc, 0), proj_q(oc, 1),
                )

            def iter_filler(it):
                if it == 0:
                    return chain(
                        proj_v(0, 0), proj_v(1, 0),
                        proj_k(0, 1), proj_v(2, 0), proj_v(3, 0),
                        proj_k(0, 2), proj_v(4, 0), proj_v(5, 0),
                        proj_k(0, 3), proj_v(6, 0), proj_v(7, 0),
                        proj_v(8, 0), proj_v(9, 0), proj_q(0, 1),
                        proj_v(10, 0), proj_v(11, 0), proj_v(12, 0),
                        proj_v(13, 0), proj_v(14, 0), proj_v(15, 0),
                    )
                if it == 1:
                    return qk_next(1)
                if it == 2:
                    return qk_next(2)
                if it == 3:
                    return chain(
                        proj_v(0, 1), proj_v(1, 1), proj_v(2, 1),
                        proj_v(3, 1), proj_v(4, 1), proj_v(5, 1),
                    )
                if it == 4:
                    return qk_next(3)
                if it == 5:
                    return chain(proj_o_dr(0), proj_v(6, 1), proj_v(7, 1))
                if it == 6:
                    return qk_next(4)
                if it == 7:
                    return chain(
                        proj_v(8, 1), proj_v(9, 1), proj_v(10, 1),
                        proj_v(11, 1), proj_v(12, 1), proj_v(13, 1),
                    )
                if it == 8:
                    return qk_next(5)
                if it == 9:
                    return chain(proj_o_dr(1), proj_v(14, 1), proj_v(15, 1))
                if it == 10:
                    return qk_next(6)
                if it == 12:
                    return qk_next(7)
                if it == 13:
                    return proj_o_dr(2)
                if it == 14:
                    return chain(
                        pads(9),  # pair-6 sqc1 norms finish at step 2
                        proj_o_single(6),
                        resid_dma(0),
                    )
                if it == 15:
                    return chain(
                        pads(12),  # pair-7 sqc0 norms finish at step 2
                        proj_o_single(7, s2_range=range(4)),
                        resid_dma(1), resid_dma(2),
                    )
                return pads(0)

            # --- prologue: just enough for pair 0 to start -----------------
            for _ in proj_q(0, 0):
                pass
            for _ in proj_k(0, 0):
                pass

            # --- attention main loop ---------------------------------------
            pending_a = []  # last-ctx + softmax-denominator recs
            pending_b = []  # R broadcast matmuls + ctxT normalize

            def emit_pair(hp, sqc, it, q_next, budget):
                cps = [
                    cpool.tile([P, 512], F32, tag="pcx", name="ps_ctx")
                    for _ in range(2)
                ]

                def emit_ctx(kt, e2):
                    for h in range(2):
                        nc.tensor.matmul(
                            cps[h][0 : HD + 1, :],
                            v_sb[:, 2 * kt : 2 * kt + 2, 2 * hp + h, :],
                            e2[:, :, h * 512 : (h + 1) * 512],
                            start=(kt == 0),
                            stop=(kt == NKT - 1),
                            perf_mode=DR,
                        )

                prev = None
                for kt in range(NKT):
                    e2 = epool.tile([P, 2, 1024], FP8, tag="e2", name="e2")
                    for j in range(2):
                        kvc = 2 * kt + j
                        if kvc == 1:
                            while pending_a:
                                pending_a.pop(0)()
                        elif kvc == 2:
                            while pending_b:
                                pending_b.pop(0)()
                        sp = spool.tile([P, 1024], F32, tag="psc", name="ps_sc")
                        for h in range(2):
                            nc.tensor.matmul(
                                sp[:, h * 512 : (h + 1) * 512],
                                ktall[
                                    h * HD : (h + 1) * HD, hp, kvc * P : (kvc + 1) * P
                                ],
                                qt[
                                    h * HD : (h + 1) * HD,
                                    hp,
                                    sqc * 512 : (sqc + 1) * 512,
                                ],
                                start=True,
                                stop=True,
                            )
                        for _ in range(budget):
                            q_next()
                        nc.scalar.activation(e2[:, j, :], sp, AF.Exp, scale=EXPS)
                    if prev is not None:
                        emit_ctx(kt - 1, prev)
                    prev = e2
                last_e2 = prev

                def deferred_tail(emit_ctx=emit_ctx, cps=cps, last_e2=last_e2):
                    # final ctx pair + softmax denominators: 1/x as exp(-ln(x))
                    # on ACT (a [1,512] DVE reciprocal costs 3.4us)
                    emit_ctx(NKT - 1, last_e2)
                    recs = []
                    for h in range(2):
                        lnt = smpool.tile([1, 512], F32, tag="sm", name="lnt")
                        nc.scalar.activation(lnt, cps[h][HD : HD + 1, :], AF.Ln)
                        rec = smpool.tile([1, 512], BF16, tag="sm", name="rec")
                        nc.scalar.activation(rec, lnt, AF.Exp, scale=-1.0)
                        recs.append(rec)

                    def deferred_norm(hp=hp, sqc=sqc, cps=cps, recs=recs):
                        for h in range(2):
                            rp = mpool.tile([P, 512], F32, tag="pmi", name="ps_r")
                            nc.tensor.matmul(
                                rp[0:HD, :], ones64, recs[h], start=True, stop=True
                            )
                            r_sb = smpool.tile([HD, 512], F32, tag="sm", name="r_sb")
                            nc.vector.tensor_copy(r_sb, rp[0:HD, :])
                            dst = ctxT[
                                h * HD : (h + 1) * HD, hp, sqc * 512 : (sqc + 1) * 512
                            ]
                            if h == 0:
                                nc.vector.tensor_tensor(
                                    dst, cps[h][0:HD, :], r_sb, OP.mult
                                )
                            else:
                                stg = smpool.tile([HD, 512], FP8, tag="sm", name="stg")
                                nc.vector.tensor_tensor(
                                    stg, cps[h][0:HD, :], r_sb, OP.mult
                                )
                                nc.sync.dma_start(dst, stg)

                    pending_b.append(deferred_norm)

                pending_a.append(deferred_tail)

            work_q = []

            def q_next():
                while work_q:
                    if next(work_q[0], "done") == "done":
                        work_q.pop(0)
                    else:
                        return

            for it in range(16):
                hp, sqc = divmod(it, 2)
                budget = 7 if it == 0 else 2
                work_q.append(iter_filler(it))
                emit_pair(hp, sqc, it, q_next, budget)
            while work_q:  # drain leftovers
                q_next()

            while pending_a:
                pending_a.pop(0)()
            while pending_b:
                pending_b.pop(0)()
            for _ in proj_o_single(7, s2_range=range(4, KC)):
                pass

            # --- RMSNorm epilogue (rs prefetch pipelined 3 deep) -----------
            for c in range(KC):
                if c + 3 < KC:
                    for _ in resid_dma(c + 3):
                        pass
                rs = resid_tiles[c]
                h_sb = hpool.tile([P, H], F32, tag="hsb", name="h_sb")
                nc.gpsimd.tensor_tensor(h_sb, accum_o[:, c, :], rs, OP.add)
                sq = sqpool.tile([P, H], F32, tag="sq", name="sq_scr")
                ss = tpool.tile([P, 1], F32, tag="tiny", name="ss")
                nc.vector.tensor_tensor(sq, h_sb, h_sb, OP.mult)
                nc.vector.tensor_reduce(ss, sq, axis=mybir.AxisListType.X, op=OP.add)
                sr = tpool.tile([P, 1], F32, tag="tiny", name="sr")
                nc.scalar.activation(sr, ss, AF.Sqrt, scale=1.0 / H, bias=eps_sb)
                rr = tpool.tile([P, 1], F32, tag="tiny", name="rr")
                nc.vector.reciprocal(rr, sr)
                nc.vector.tensor_scalar_mul(h_sb, h_sb, rr)
                nc.vector.tensor_tensor(rs, h_sb, gam_sb, OP.mult)
                nc.sync.dma_start(out[c * P : (c + 1) * P, :], rs)

    if split_waits:
        _split_sync_waits(nc)
    return nc


_NC = None


def _get_nc():
    global _NC
    if _NC is None:
        _NC = build_core_kernel()
    return _NC


def _fp8(a):
    return np.clip(a, -240.0, 240.0).astype(ml_dtypes.float8_e4m3)


def make_in_maps(hidden_states, keyvalue_states, Wq, bq, Wk, bk, Wv, bv, Wo, bo, gamma):
    f = np.float32
    hidden_states = np.asarray(hidden_states, f)
    keyvalue_states = np.asarray(keyvalue_states, f)
    shared = {
        "wqT": _fp8(np.asarray(Wq, f).T * WS),
        "wkT": _fp8(np.asarray(Wk, f).T * WS),
        "wvT": _fp8(np.asarray(Wv, f).T * WS),
        "woT": _fp8(np.asarray(Wo, f).T * WS),
        "bqc": np.ascontiguousarray(np.asarray(bq, f).reshape(KC, P).T * WS),
        "bkc": np.ascontiguousarray(np.asarray(bk, f).reshape(KC, P).T * WS),
        "bvr": np.ascontiguousarray(np.tile(np.asarray(bv, f) * WS, (P, 1))),
        "gam": np.ascontiguousarray(np.tile(np.asarray(gamma, f), (P, 1))),
    }
    bo = np.asarray(bo, f)
    in_maps = []
    for core in range(N_CORES):
        b, half = divmod(core, 2)
        hq = hidden_states[b, half * SQL : (half + 1) * SQL, :]
        m = dict(shared)
        m["xqT"] = _fp8(hq.T)
        m["xkvT"] = _fp8(keyvalue_states[b].T)
        m["resid"] = np.ascontiguousarray((hq + bo) * RS)
        in_maps.append(m)
    return in_maps


def _run(in_maps, trace=False, **kwargs):
    nc = _get_nc()
    return bass_utils.run_bass_kernel_spmd(
        nc, in_maps, core_ids=list(range(N_CORES)), trace=trace, **kwargs
    )


def _assemble(res):
    out = np.empty((B, SQ, H), np.float32)
    for core in range(N_CORES):
        b, half = divmod(core, 2)
        out[b, half * SQL : (half + 1) * SQL, :] = res.results[core]["out"]
    return out


def kernel(hidden_states, keyvalue_states, Wq, bq, Wk, bk, Wv, bv, Wo, bo, gamma):
    in_maps = make_in_maps(
        hidden_states, keyvalue_states, Wq, bq, Wk, bk, Wv, bv, Wo, bo, gamma
    )
    return _assemble(_run(in_maps))
